# revision 16
# baseline (speedup 1.0000x reference)
"""Multi-head attention block (nn_AttentionBlock) on 8 Trainium2 NeuronCores.

Reference computation (fp32):
    qkv = x @ w_qkv;  q,k,v = split(qkv)
    per head: att = softmax(q @ k.T / 8) @ v
    out = concat_heads(att) @ w_out
Shapes: x [4, 2048, 1024], w_qkv [1024, 3072], w_out [1024, 1024], 16 heads.

Sharding: batch x head-half. Core c handles batch b = c//2 and heads
h0 = (c%2)*8 .. h0+8. Each core computes its 8 heads' attention and a partial
output projection (row-shard of w_out); host sums the two partials per batch.

Per-core kernel (all heavy matmuls bf16 with fp32 PSUM accumulation):
  P1: qkT[n, s] = w_qk.T @ x.T (weights stationary, xT moving) -> q/k heads
      transposed in SBUF; v[t, d] = x @ w_v in natural layout with an
      appended ones column per head (v_aug).
  P2 per head pair (2 heads share a 128-partition tile, K=64 row-packed
      matmuls at base partitions 0/64):
      ST[t, s] = exp((k_t . q_s)/8)  via TensorE + ScalarE exp (psum->sbuf
      bf16), no max subtraction (logits are O(5), exp is safe in fp32).
      PV: att_aug[d|1, s] = v_aug.T @ p accumulated over 16 t-chunks; row 64
      is r[s] = sum_t p. rinv broadcast via K=1 ones matmul + reciprocal;
      normalize att rows with a tensor-tensor multiply.
  P3: partial[s, e] = att.T @ w_out accumulated over the 4 head pairs.
"""

import sys

sys.path.insert(0, "/opt/trn_rl_repo")

import numpy as np
import ml_dtypes

import concourse.mybir as mybir
import concourse.tile as tile
from concourse import bacc
from concourse import bass_utils
from concourse import dve_ops as _dve_ops
from concourse.dve_spec import (
    Spec, Src0, C0, C1, C2, C3, sq, lower, _spill_c3_to_src1, _has_src1,
)
from concourse.dve_uop import DveOpSpec

F32 = mybir.dt.float32
F32R = mybir.dt.float32r
BF16 = mybir.dt.bfloat16

# ---------------------------------------------------------------------------
# Custom DVE exp: exp(x/8) = P(x)^64 with P a cubic ~ e^(x/512).
# Pass 1 (EXP_P3_ANT): Horner cubic, coefficients pre-scaled by s=1/512.
# Pass 2 (POW64_ANT): six chained squares.
# Max rel err 5.5e-5 over x in [-56, 56] (logits here are O(6) * 8).
# Registered into dve_ops at import so the per-NEFF DVE table includes them.
_EXP_S = 0.125 / 64.0
_EXP_COEF = (
    0.9999992332857001,                      # c0
    0.9999996170667287 * _EXP_S,             # c1*s
    0.5005056061615765 * _EXP_S ** 2,        # c2*s^2
    0.16679299496749722 * _EXP_S ** 3,       # c3*s^3  (via C3 -> in1 latch)
)


def _np_ref_exp_p3(in0, in1, s0, s1, imm2):
    c3 = np.asarray(in1, dtype=np.float32).reshape(-1)[0]
    x = np.asarray(in0, dtype=np.float32)
    return (((c3 * x + imm2) * x + s1) * x + s0).astype(np.float32)


def _np_ref_pow64(in0, in1, s0, s1, imm2):
    x = np.asarray(in0, dtype=np.float32)
    for _ in range(6):
        x = x * x
    return x.astype(np.float32)


def _register_exp_dve_ops():
    if "EXP_P3_ANT" in _dve_ops._SUB_OPCODE_FOR_NAME:
        return ([op for op in _dve_ops.OPS if op.name == "EXP_P3_ANT"][0],
                [op for op in _dve_ops.OPS if op.name == "POW64_ANT"][0])
    specs = [
        ("EXP_P3_ANT",
         Spec(body=_spill_c3_to_src1(((C3 * Src0 + C2) * Src0 + C1) * Src0
                                     + C0),
              reference=_np_ref_exp_p3)),
        ("POW64_ANT",
         Spec(body=sq(sq(sq(sq(sq(sq(Src0)))))), reference=_np_ref_pow64)),
    ]
    made = []
    for name, spec in specs:
        row = _dve_ops._CUSTOM_DVE_ROW_BASE + len(_dve_ops.OPS)
        shas = {}
        for ver in ("v3", "v4"):
            dos = DveOpSpec(name=name, opcode=row, uops=lower(spec, ver=ver),
                            rd1_en=_has_src1(spec))
            shas[ver] = dos.sha(ver)
        op = _dve_ops.DveOp(name, spec, subdim=False, uops_sha=shas)
        _dve_ops.OPS.append(op)
        _dve_ops._SUB_OPCODE_FOR_NAME[name] = row
        _dve_ops.CUSTOM_DVE_SPECS[name] = spec
        made.append(op)
    return tuple(made)


EXP_P3_ANT, POW64_ANT = _register_exp_dve_ops()

S = 2048          # sequence length
D = 1024          # embed dim
DH = 64           # head dim
NH = 8            # heads per core
NPAIR = 4         # head pairs per core
KC = D // 128     # contraction chunks for phase 1
SC = S // 512     # 512-wide free chunks of the sequence
TT = S // 128     # 128-row t tiles of the sequence
N_CORES = 8

_CACHED_NC = None


def build_nc():
    nc = bacc.Bacc("TRN2", target_bir_lowering=False, debug=False,
                   num_devices=N_CORES)

    xt = nc.dram_tensor("xt", [D, S], BF16, kind="ExternalInput").ap()
    wqk = nc.dram_tensor("wqk", [D, D], BF16, kind="ExternalInput").ap()
    wv = nc.dram_tensor("wv", [D, NH * DH], BF16, kind="ExternalInput").ap()
    wout = nc.dram_tensor("wout", [NH * DH, D], BF16, kind="ExternalInput").ap()
    out_p = nc.dram_tensor("out_p", [S, D], F32, kind="ExternalOutput").ap()
    warm = nc.dram_tensor("warm", [1, 512], F32, kind="ExternalOutput").ap()

    with tile.TileContext(nc) as tc:
        with (
            tc.tile_pool(name="sbc", bufs=1) as sbc,        # constants
            tc.tile_pool(name="sbin", bufs=1) as sbin,      # phase-1 inputs
            tc.tile_pool(name="sbqk", bufs=1) as sbqk,      # qkT persistent
            tc.tile_pool(name="sbv", bufs=1) as sbv,        # v_aug persistent
            tc.tile_pool(name="sbp", bufs=10) as sbp,        # exp(ST) tiles
            tc.tile_pool(name="sbr", bufs=4) as sbr,        # r chunks (f32r)
            tc.tile_pool(name="sbrv", bufs=2) as sbrv,      # rinv chunks
            tc.tile_pool(name="sbst", bufs=2) as sbst,      # B-head att stage
            tc.tile_pool(name="sbdp", bufs=2) as sbdp,      # dup q/k scratch
            tc.tile_pool(name="sbat", bufs=1) as sbat,      # att pairs
            tc.tile_pool(name="sbo", bufs=4) as sbo,        # out evac
            tc.tile_pool(name="psb", bufs=2, space="PSUM") as psb,   # [128,1024]
            tc.tile_pool(name="psa", bufs=3, space="PSUM") as psa,   # [128,512]
            tc.tile_pool(name="psc", bufs=1, space="PSUM") as psc,   # [128,512]
        ):
            # ---------- constants ----------
            ones_f = sbc.tile([128, 128], F32, name="ones_f")
            nc.gpsimd.memset(ones_f[:], 1.0)
            ones_r = sbc.tile([128, 128], F32R, name="ones_r")
            nc.vector.tensor_copy(ones_r[:], ones_f[:])

            # ---------- input DMA ----------
            xt_t = [sbin.tile([128, S], BF16, name=f"xt{k}", tag=f"xt{k}")
                    for k in range(KC)]
            wqk_t = [sbin.tile([128, D], BF16, name=f"wqk{k}", tag=f"wqk{k}")
                     for k in range(KC)]
            wv_t = [sbin.tile([128, NH * DH], BF16, name=f"wv{k}", tag=f"wv{k}")
                    for k in range(KC)]
            wout_t = [sbin.tile([128, D], BF16, name=f"wout{p}", tag=f"wout{p}")
                      for p in range(NPAIR)]
            # wv first (phase 1b needs it in full), xt s-chunk-major (phase 1b
            # consumes s-chunks in order), wqk early (phase 1a gates phase 2),
            # wout last. Spread across three DMA queues (sync HW, vector SW,
            # gpsimd SW) so the load isn't serialized on one ring.
            for k in range(KC):
                nc.gpsimd.dma_start(wv_t[k][:], wv[k * 128:(k + 1) * 128, :])
            for sc in range(SC):
                for k in range(KC):
                    nc.sync.dma_start(
                        xt_t[k][:, sc * 512:(sc + 1) * 512],
                        xt[k * 128:(k + 1) * 128, sc * 512:(sc + 1) * 512])
                if sc == 1:
                    # wqk after the first two s-chunks: phase 1a (pair 0)
                    # needs it at ~the same time as xt chunk 2 lands
                    for k in range(KC):
                        nc.sync.dma_start(wqk_t[k][:],
                                          wqk[k * 128:(k + 1) * 128, :])
            for p in range(NPAIR):
                nc.sync.dma_start(wout_t[p][:], wout[p * 128:(p + 1) * 128, :])

            # ---------- persistent intermediates ----------
            # qkT rows: tiles 0..3 = qT head pairs, 4..7 = kT head pairs
            qk_t = [sbqk.tile([128, S], BF16, name=f"qk{n}", tag=f"qk{n}")
                    for n in range(2 * NPAIR)]
            # v_aug: [t, 8*(64+1)]; per head 64 v columns then a ones column
            v_t = [sbv.tile([128, NH * (DH + 1)], BF16, name=f"v{t}", tag=f"v{t}")
                   for t in range(TT)]
            att_t = [sbat.tile([128, S], BF16, name=f"att{p}", tag=f"att{p}")
                     for p in range(NPAIR)]

            # ---------- phase 1b: v (natural layout) ----------
            def emit_v(tb):
                ps = psb.tile([128, 1024], F32, name=f"psv{tb}", tag="psb")
                for k in range(KC):
                    nc.tensor.matmul(
                        ps[:, 0:NH * DH],
                        xt_t[k][:, tb * 128:(tb + 1) * 128],
                        wv_t[k][:],
                        start=(k == 0),
                        stop=(k == KC - 1),
                    )
                nc.gpsimd.memset(v_t[tb][:], 1.0)
                nc.vector.tensor_copy(
                    v_t[tb][:].rearrange("p (h e) -> p h e", e=DH + 1)[:, :, 0:DH],
                    ps[:, 0:NH * DH].rearrange("p (h e) -> p h e", e=DH),
                )

            for tb in range(TT):
                emit_v(tb)

            head_order = [1, 0, 3, 2, 5, 4, 7, 6]   # B before A per pair
            dup_q = {}
            dup_k = {}

            def emit_dup_half(h, sh):
                """Duplicate head h's q/k rows (s-columns sh*1024:+1024) to
                both partition halves so two t-tiles' ST matmuls row-pack
                (bases 0 and 64)."""
                pr, half = h // 2, h % 2
                r0, r1 = half * 64, half * 64 + 64
                if h not in dup_q:
                    dup_q[h] = sbdp.tile([128, S], BF16, name=f"qd{h}",
                                         tag="qd")
                    dup_k[h] = sbdp.tile([128, S], BF16, name=f"kd{h}",
                                         tag="kd")
                cols = slice(sh * 1024, (sh + 1) * 1024)
                for dst0, eng in ((0, nc.sync), (64, nc.gpsimd)):
                    eng.dma_start(dup_q[h][dst0:dst0 + 64, cols],
                                  qk_t[pr][r0:r1, cols])
                    eng.dma_start(dup_k[h][dst0:dst0 + 64, cols],
                                  qk_t[NPAIR + pr][r0:r1, cols])

            def emit_dup(h):
                emit_dup_half(h, 0)
                emit_dup_half(h, 1)

            # ---------- phase 1a: qT / kT for head pair 0 (serial) ----------
            p1a_pair0 = ((NPAIR, 0), (0, 0), (NPAIR, 1), (0, 1))
            dup_after = {(0, 0): 0, (0, 1): 1}   # dup half after these units
            for nb, scp in p1a_pair0:
                ps = psb.tile([128, 1024], F32, name=f"psq{nb}_{scp}",
                              tag="psb")
                for half1 in range(2):
                    sc = 2 * scp + half1
                    for k in range(KC):
                        nc.tensor.matmul(
                            ps[:, half1 * 512:(half1 + 1) * 512],
                            wqk_t[k][:, nb * 128:(nb + 1) * 128],
                            xt_t[k][:, sc * 512:(sc + 1) * 512],
                            start=(k == 0),
                            stop=(k == KC - 1),
                        )
                nc.vector.tensor_copy(
                    qk_t[nb][:, scp * 1024:(scp + 1) * 1024], ps[:])
                if (nb, scp) in dup_after:
                    emit_dup_half(head_order[0], dup_after[(nb, scp)])

            # ---------- phase 2: attention (s-halved, software-pipelined) --
            # Flat stream over (head, s-half pass, t-tile). ScalarE exp is
            # the bottleneck; TensorE rides ~1 step of ST + lagged PV + one
            # drip-fed phase-1a matmul per step (pairs 1-3 of qkT computed
            # inside the attention stream through a dedicated PSUM bank).
            LAG = 3

            # injected phase-1a work: one unit = one (nb, s-chunk) 8-matmul
            # accumulation + cast, split into single-matmul thunks
            def make_inject_thunks():
                # (nb, sc) units, front-loaded: the dup-DMA prefetch for a
                # pair's heads reads the whole qkT tile, so pair p must be
                # complete ~8 tp-steps before its first head starts
                order = []
                releases = []
                u = 0
                for bi, base in enumerate((1, 2, 3)):
                    qn, kn = base, NPAIR + base
                    for sc in range(SC):
                        for nbx in (qn, kn):
                            order.append((nbx, sc))
                            # pair 1 front-loaded (its dup prefetch reads the
                            # whole tile at step 24); pairs 2/3 stretched to
                            # keep the PE fed through step ~85
                            if bi == 0:
                                releases.append(2 * u)
                            else:
                                releases.append(16 * bi + 4 * (u - 8 * bi))
                            u += 1
                thunks = []
                rel = []
                for u, (nb, sc) in enumerate(order):
                    state = {}

                    def mk_mm(k, nb=nb, sc=sc, state=state):
                        def run():
                            if "ps" not in state:
                                state["ps"] = psc.tile(
                                    [128, 512], F32,
                                    name=f"psi{nb}_{sc}", tag="psc")
                            nc.tensor.matmul(
                                state["ps"][:],
                                wqk_t[k][:, nb * 128:(nb + 1) * 128],
                                xt_t[k][:, sc * 512:(sc + 1) * 512],
                                start=(k == 0),
                                stop=(k == KC - 1),
                            )
                        return run

                    def mk_evac(nb=nb, sc=sc, state=state):
                        def run():
                            nc.vector.tensor_copy(
                                qk_t[nb][:, sc * 512:(sc + 1) * 512],
                                state["ps"][:])
                        return run

                    for k in range(KC):
                        thunks.append(mk_mm(k))
                        rel.append(releases[u] + k)
                    thunks.append(mk_evac())
                    rel.append(releases[u] + KC)
                return thunks, rel

            inject, inj_rel = make_inject_thunks()
            inj_pos = 0

            # HAM warm-keeper: harmless matmuls accumulating into a scratch
            # PSUM tile (flushed to a tiny scratch output so DCE keeps them).
            warm_state = {"n": 0, "ps": None}

            def emit_warm():
                if warm_state["ps"] is None:
                    warm_state["ps"] = psc.tile([128, 512], F32,
                                                name="pswarm", tag="psc")
                nc.tensor.matmul(
                    warm_state["ps"][:, 0:128],
                    wqk_t[0][:, 0:128],
                    xt_t[0][:, 0:128],
                    start=(warm_state["n"] == 0),
                    stop=False,
                    skip_group_check=True,
                )
                warm_state["n"] += 1

            def flush_warm():
                if warm_state["ps"] is None:
                    return
                nc.tensor.matmul(
                    warm_state["ps"][:, 0:128],
                    wqk_t[0][:, 0:128],
                    xt_t[0][:, 0:128],
                    start=False,
                    stop=True,
                    skip_group_check=True,
                )
                wsb = sbo.tile([1, 512], F32, name="warm_sb", tag="warm_sb")
                nc.vector.tensor_copy(wsb[:, 0:128], warm_state["ps"][0:1, 0:128])
                nc.sync.dma_start(warm[:, 0:128], wsb[:, 0:128])
                warm_state["ps"] = None

            p_tiles = {}    # (h, pa, tt) -> p tile
            accs = {}       # (h, pa) -> [2 psum accumulators]
            r_cs = {}       # (h, pa) -> [2 r tiles]
            stages = {}     # h -> B-head staging tile

            def retire(hp, pap, tpp):
                """Emit PV for t-tiles 2*tpp, 2*tpp+1; after the last pair
                emit the normalization tail inline."""
                h = hp
                if tpp == 0:
                    accs[(hp, pap)] = [
                        psa.tile([128, 512], F32, name=f"pv{hp}_{pap}_{cc}",
                                 tag="psa")
                        for cc in range(2)]
                acc = accs[(hp, pap)]
                for ttp in (2 * tpp, 2 * tpp + 1):
                    pt = p_tiles.pop((hp, pap, ttp))
                    for cc in range(2):
                        nc.tensor.matmul(
                            acc[cc][0:DH + 1, :],
                            v_t[ttp][:, h * (DH + 1):(h + 1) * (DH + 1)],
                            pt[:, cc * 512:(cc + 1) * 512],
                            start=(ttp == 0),
                            stop=(ttp == TT - 1),
                        )
                if tpp == TT // 2 - 1:
                    pr, half = hp // 2, hp % 2
                    if half == 0:
                        dst = att_t[pr]
                    else:
                        if pap == 0:
                            stages[hp] = sbst.tile([64, S], BF16,
                                                   name=f"bst{hp}",
                                                   tag="bstage")
                        dst = stages[hp]
                    for cc in range(2):
                        cg = pap * 2 + cc
                        r = sbr.tile([128, 512], F32R,
                                     name=f"r{hp}_{pap}_{cc}", tag="r")
                        nc.vector.tensor_copy(r[64:65, :], acc[cc][64:65, :])
                        rb = psa.tile([128, 512], F32,
                                      name=f"rb{hp}_{pap}_{cc}", tag="psa")
                        nc.tensor.matmul(
                            rb[0:64, :],
                            ones_r[64:65, 0:64],
                            r[64:65, :],
                            start=True,
                            stop=True,
                        )
                        rinv = sbrv.tile([64, 512], F32,
                                         name=f"rinv{hp}_{pap}_{cc}",
                                         tag="rinv")
                        nc.vector.reciprocal_approx_fast(rinv[:], rb[0:64, :])
                        nc.vector.tensor_mul(
                            dst[0:64, cg * 512:(cg + 1) * 512],
                            acc[cc][0:64, :],
                            rinv[:],
                        )
                    if half == 1 and pap == 1:
                        nc.gpsimd.dma_start(att_t[pr][64:128, :],
                                            stages[hp][0:64, :])

            seq = [(h, pa, tp)
                   for h in head_order for pa in range(2)
                   for tp in range(TT // 2)]
            for i, (h, pa, tp) in enumerate(seq):
                qd, kd = dup_q[h], dup_k[h]
                tt0, tt1 = 2 * tp, 2 * tp + 1

                # emit PV/injection BEFORE the ST quad: the scheduler pops
                # ready work in priority (program) order, so nothing can
                # interleave into the row-packed A/B matmul pairs
                if pa == 1 and tp == 4:
                    # prefetch dup tiles for the next head (12 steps ahead;
                    # late enough that injected qkT writes precede the read)
                    hi = head_order.index(h)
                    if hi + 1 < len(head_order):
                        emit_dup(head_order[hi + 1])
                if i >= LAG:
                    retire(*seq[i - LAG])
                ran = 0
                while (inj_pos < len(inject) and inj_rel[inj_pos] <= i
                       and ran < 4):
                    inject[inj_pos]()
                    inj_pos += 1
                    ran += 1
                if ran == 0:
                    emit_warm()

                ptA = sbp.tile([128, 1024], BF16, name=f"p{h}_{pa}_{tt0}",
                               tag="p")
                ptB = sbp.tile([128, 1024], BF16, name=f"p{h}_{pa}_{tt1}",
                               tag="p")
                p_tiles[(h, pa, tt0)] = ptA
                p_tiles[(h, pa, tt1)] = ptB
                psA = psb.tile([128, 1024], F32, name=f"pstA{h}_{pa}_{tp}",
                               tag="psb")
                psB = psb.tile([128, 1024], F32, name=f"pstB{h}_{pa}_{tp}",
                               tag="psb")
                for j in range(2):
                    nc.tensor.matmul(
                        psA[:, j * 512:(j + 1) * 512],
                        kd[0:64, tt0 * 128:(tt0 + 1) * 128],
                        qd[0:64, pa * 1024 + j * 512:pa * 1024 + (j + 1) * 512],
                        start=True,
                        stop=True,
                    )
                    nc.tensor.matmul(
                        psB[:, j * 512:(j + 1) * 512],
                        kd[64:128, tt1 * 128:(tt1 + 1) * 128],
                        qd[64:128, pa * 1024 + j * 512:pa * 1024 + (j + 1) * 512],
                        start=True,
                        stop=True,
                    )
                nc.scalar.activation(
                    ptA[:], psA[:], mybir.ActivationFunctionType.Exp,
                    scale=0.125)
                nc.scalar.activation(
                    ptB[:], psB[:], mybir.ActivationFunctionType.Exp,
                    scale=0.125)
            while inj_pos < len(inject):
                inject[inj_pos]()
                inj_pos += 1
            for i in range(len(seq) - LAG, len(seq)):
                retire(*seq[i])
                for _ in range(6):
                    emit_warm()
            for _ in range(12):
                emit_warm()
            flush_warm()

            # ---------- phase 3: output projection ----------
            # attention pools are idle now; rotate psum groups across all
            # three pools for deeper pipelining
            for sb_i in range(TT):
                for e in range(2):
                    sel = (2 * sb_i + e) % 4
                    if sel < 2:
                        ps = psa.tile([128, 512], F32, name=f"po{sb_i}_{e}",
                                      tag="psa")
                    elif sel == 2:
                        psw = psb.tile([128, 1024], F32, name=f"po{sb_i}_{e}",
                                       tag="psb")
                        ps = psw[:, 0:512]
                    else:
                        ps = psc.tile([128, 512], F32, name=f"po{sb_i}_{e}",
                                      tag="psc")
                    for pr in range(NPAIR):
                        nc.tensor.matmul(
                            ps[:],
                            att_t[pr][:, sb_i * 128:(sb_i + 1) * 128],
                            wout_t[pr][:, e * 512:(e + 1) * 512],
                            start=(pr == 0),
                            stop=(pr == NPAIR - 1),
                        )
                    o = sbo.tile([128, 512], F32, name=f"o{sb_i}_{e}", tag="o")
                    nc.vector.tensor_copy(o[:], ps[:])
                    # output DMA alternates sync/scalar HW queues (ScalarE is
                    # idle once phase 2's activations are done)
                    oeng = nc.sync if (2 * sb_i + e) % 2 == 0 else nc.scalar
                    oeng.dma_start(
                        out_p[sb_i * 128:(sb_i + 1) * 128,
                              e * 512:(e + 1) * 512], o[:])

    nc.compile()
    return nc


def get_nc():
    global _CACHED_NC
    if _CACHED_NC is None:
        _CACHED_NC = build_nc()
    return _CACHED_NC


def make_in_maps(x, w_qkv, w_out):
    bf = ml_dtypes.bfloat16
    in_maps = []
    for c in range(N_CORES):
        b = c // 2
        h0 = (c % 2) * NH
        q_cols = w_qkv[:, h0 * DH:(h0 + NH) * DH]
        k_cols = w_qkv[:, D + h0 * DH:D + (h0 + NH) * DH]
        v_cols = w_qkv[:, 2 * D + h0 * DH:2 * D + (h0 + NH) * DH]
        in_maps.append({
            "xt": np.ascontiguousarray(x[b].T).astype(bf),
            "wqk": np.concatenate([q_cols, k_cols], axis=1).astype(bf),
            "wv": np.ascontiguousarray(v_cols).astype(bf),
            "wout": np.ascontiguousarray(
                w_out[h0 * DH:(h0 + NH) * DH, :]).astype(bf),
        })
    return in_maps


def run(x, w_qkv, w_out, trace=False, trace_cores=None):
    nc = get_nc()
    in_maps = make_in_maps(x, w_qkv, w_out)
    res = bass_utils.run_bass_kernel_spmd(
        nc, in_maps, core_ids=list(range(N_CORES)),
        trace=trace, trace_cores=trace_cores,
    )
    partials = [res.results[c]["out_p"] for c in range(N_CORES)]
    out = np.stack([partials[2 * b] + partials[2 * b + 1] for b in range(4)])
    return out.astype(np.float32), res


def kernel(x, w_qkv, w_out):
    out, _ = run(np.asarray(x), np.asarray(w_qkv), np.asarray(w_out))
    return out


if __name__ == "__main__":
    import tempfile
    nc = build_nc()
    with tempfile.TemporaryDirectory() as td:
        neff = bass_utils.compile_bass_kernel(nc, td)
        print("LOCAL COMPILE OK:", neff)



# revision 27
# speedup vs baseline: 1.0542x; 1.0542x over previous
"""Multi-head attention block (nn_AttentionBlock) on 8 Trainium2 NeuronCores.

Reference computation (fp32):
    qkv = x @ w_qkv;  q,k,v = split(qkv)
    per head: att = softmax(q @ k.T / 8) @ v
    out = concat_heads(att) @ w_out
Shapes: x [4, 2048, 1024], w_qkv [1024, 3072], w_out [1024, 1024], 16 heads.

Sharding: batch x head-half. Core c handles batch b = c//2 and heads
h0 = (c%2)*8 .. h0+8. Each core computes its 8 heads' attention and a partial
output projection (row-shard of w_out); host sums the two partials per batch.

Per-core kernel (all heavy matmuls bf16 with fp32 PSUM accumulation):
  P1: qkT[n, s] = w_qk.T @ x.T (weights stationary, xT moving) -> q/k heads
      transposed in SBUF; v[t, d] = x @ w_v in natural layout with an
      appended ones column per head (v_aug).
  P2 per head pair (2 heads share a 128-partition tile, K=64 row-packed
      matmuls at base partitions 0/64):
      ST[t, s] = exp((k_t . q_s)/8)  via TensorE + ScalarE exp (psum->sbuf
      bf16), no max subtraction (logits are O(5), exp is safe in fp32).
      PV: att_aug[d|1, s] = v_aug.T @ p accumulated over 16 t-chunks; row 64
      is r[s] = sum_t p. rinv broadcast via K=1 ones matmul + reciprocal;
      normalize att rows with a tensor-tensor multiply.
  P3: partial[s, e] = att.T @ w_out accumulated over the 4 head pairs.
"""

import sys

sys.path.insert(0, "/opt/trn_rl_repo")

import numpy as np
import ml_dtypes

import concourse.mybir as mybir
import concourse.tile as tile
from concourse import bacc
from concourse import bass_utils
from concourse import dve_ops as _dve_ops
from concourse.dve_spec import (
    Spec, Src0, C0, C1, C2, C3, sq, lower, _spill_c3_to_src1, _has_src1,
)
from concourse.dve_uop import DveOpSpec

F32 = mybir.dt.float32
F32R = mybir.dt.float32r
BF16 = mybir.dt.bfloat16

# ---------------------------------------------------------------------------
# Custom DVE exp: exp(x/8) = P(x)^64 with P a cubic ~ e^(x/512).
# Pass 1 (EXP_P3_ANT): Horner cubic, coefficients pre-scaled by s=1/512.
# Pass 2 (POW64_ANT): six chained squares.
# Max rel err 5.5e-5 over x in [-56, 56] (logits here are O(6) * 8).
# Registered into dve_ops at import so the per-NEFF DVE table includes them.
_EXP_S = 0.125 / 64.0
_EXP_COEF = (
    0.9999992332857001,                      # c0
    0.9999996170667287 * _EXP_S,             # c1*s
    0.5005056061615765 * _EXP_S ** 2,        # c2*s^2
    0.16679299496749722 * _EXP_S ** 3,       # c3*s^3  (via C3 -> in1 latch)
)


def _np_ref_exp_p3(in0, in1, s0, s1, imm2):
    c3 = np.asarray(in1, dtype=np.float32).reshape(-1)[0]
    x = np.asarray(in0, dtype=np.float32)
    return (((c3 * x + imm2) * x + s1) * x + s0).astype(np.float32)


def _np_ref_pow64(in0, in1, s0, s1, imm2):
    x = np.asarray(in0, dtype=np.float32)
    for _ in range(6):
        x = x * x
    return x.astype(np.float32)


def _register_exp_dve_ops():
    if "EXP_P3_ANT" in _dve_ops._SUB_OPCODE_FOR_NAME:
        return ([op for op in _dve_ops.OPS if op.name == "EXP_P3_ANT"][0],
                [op for op in _dve_ops.OPS if op.name == "POW64_ANT"][0])
    specs = [
        ("EXP_P3_ANT",
         Spec(body=_spill_c3_to_src1(((C3 * Src0 + C2) * Src0 + C1) * Src0
                                     + C0),
              reference=_np_ref_exp_p3)),
        ("POW64_ANT",
         Spec(body=sq(sq(sq(sq(sq(sq(Src0)))))), reference=_np_ref_pow64)),
    ]
    made = []
    for name, spec in specs:
        row = _dve_ops._CUSTOM_DVE_ROW_BASE + len(_dve_ops.OPS)
        shas = {}
        for ver in ("v3", "v4"):
            dos = DveOpSpec(name=name, opcode=row, uops=lower(spec, ver=ver),
                            rd1_en=_has_src1(spec))
            shas[ver] = dos.sha(ver)
        op = _dve_ops.DveOp(name, spec, subdim=False, uops_sha=shas)
        _dve_ops.OPS.append(op)
        _dve_ops._SUB_OPCODE_FOR_NAME[name] = row
        _dve_ops.CUSTOM_DVE_SPECS[name] = spec
        made.append(op)
    return tuple(made)


EXP_P3_ANT, POW64_ANT = _register_exp_dve_ops()

S = 2048          # sequence length
D = 1024          # embed dim
DH = 64           # head dim
NH = 8            # heads per core
NPAIR = 4         # head pairs per core
KC = D // 128     # contraction chunks for phase 1
SC = S // 512     # 512-wide free chunks of the sequence
TT = S // 128     # 128-row t tiles of the sequence
N_CORES = 8
# 1 of every N exp half-tiles runs on the DVE; 0 disables (the custom-DVE
# exp path crashes NRT on this runtime build — kept for reference)
DVE_EXP_RATIO = 0

_CACHED_NC = None


def build_nc():
    nc = bacc.Bacc("TRN2", target_bir_lowering=False, debug=False,
                   num_devices=N_CORES)

    xt = nc.dram_tensor("xt", [D, S], BF16, kind="ExternalInput").ap()
    wqk = nc.dram_tensor("wqk", [D, D], BF16, kind="ExternalInput").ap()
    wv = nc.dram_tensor("wv", [D, NH * DH], BF16, kind="ExternalInput").ap()
    wout = nc.dram_tensor("wout", [NH * DH, D], BF16, kind="ExternalInput").ap()
    out_p = nc.dram_tensor("out_p", [S, D], F32, kind="ExternalOutput").ap()
    warm = nc.dram_tensor("warm", [1, 512], F32, kind="ExternalOutput").ap()

    with tile.TileContext(nc) as tc:
        with (
            tc.tile_pool(name="sbc", bufs=1) as sbc,        # constants
            tc.tile_pool(name="sbin", bufs=1) as sbin,      # phase-1 inputs
            tc.tile_pool(name="sbqk", bufs=1) as sbqk,      # qkT persistent
            tc.tile_pool(name="sbv", bufs=1) as sbv,        # v_aug persistent
            tc.tile_pool(name="sbp", bufs=10) as sbp,        # exp(ST) tiles
            tc.tile_pool(name="sbr", bufs=4) as sbr,        # r chunks (f32r)
            tc.tile_pool(name="sbrv", bufs=2) as sbrv,      # rinv chunks
            tc.tile_pool(name="sbst", bufs=2) as sbst,      # B-head att stage
            tc.tile_pool(name="sbdp", bufs=2) as sbdp,      # dup q/k scratch
            tc.tile_pool(name="sbat", bufs=1) as sbat,      # att pairs
            tc.tile_pool(name="sbo", bufs=4) as sbo,        # out evac
            tc.tile_pool(name="sbxp", bufs=1) as sbxp,      # DVE-exp scratch
            tc.tile_pool(name="psb", bufs=2, space="PSUM") as psb,   # [128,1024]
            tc.tile_pool(name="psa", bufs=3, space="PSUM") as psa,   # [128,512]
            tc.tile_pool(name="psc", bufs=1, space="PSUM") as psc,   # [128,512]
        ):
            # ---------- constants ----------
            ones_f = sbc.tile([128, 128], F32, name="ones_f")
            nc.gpsimd.memset(ones_f[:], 1.0)
            ones_r = sbc.tile([128, 128], F32R, name="ones_r")
            nc.vector.tensor_copy(ones_r[:], ones_f[:])
            c3col = sbc.tile([128, 1], F32, name="c3col")
            nc.gpsimd.memset(c3col[:], _EXP_COEF[3])

            # ---------- input DMA ----------
            xt_t = [sbin.tile([128, S], BF16, name=f"xt{k}", tag=f"xt{k}")
                    for k in range(KC)]
            wqk_t = [sbin.tile([128, D], BF16, name=f"wqk{k}", tag=f"wqk{k}")
                     for k in range(KC)]
            wv_t = [sbin.tile([128, NH * DH], BF16, name=f"wv{k}", tag=f"wv{k}")
                    for k in range(KC)]
            wout_t = [sbin.tile([128, D], BF16, name=f"wout{p}", tag=f"wout{p}")
                      for p in range(NPAIR)]
            # wv first (phase 1b needs it in full), xt s-chunk-major (phase 1b
            # consumes s-chunks in order), wqk early (phase 1a gates phase 2),
            # wout last. Spread across three DMA queues (sync HW, vector SW,
            # gpsimd SW) so the load isn't serialized on one ring.
            for k in range(KC):
                nc.gpsimd.dma_start(wv_t[k][:], wv[k * 128:(k + 1) * 128, :])
            for sc in range(SC):
                for k in range(KC):
                    nc.sync.dma_start(
                        xt_t[k][:, sc * 512:(sc + 1) * 512],
                        xt[k * 128:(k + 1) * 128, sc * 512:(sc + 1) * 512])
            for k in range(KC):
                nc.sync.dma_start(wqk_t[k][:], wqk[k * 128:(k + 1) * 128, :])
            for p in range(NPAIR):
                nc.sync.dma_start(wout_t[p][:], wout[p * 128:(p + 1) * 128, :])

            # ---------- persistent intermediates ----------
            # qkT rows: tiles 0..3 = qT head pairs, 4..7 = kT head pairs
            qk_t = [sbqk.tile([128, S], BF16, name=f"qk{n}", tag=f"qk{n}")
                    for n in range(2 * NPAIR)]
            # v_aug: [t, 8*(64+1)]; per head 64 v columns then a ones column
            v_t = [sbv.tile([128, NH * (DH + 1)], BF16, name=f"v{t}", tag=f"v{t}")
                   for t in range(TT)]
            att_t = [sbat.tile([128, S], BF16, name=f"att{p}", tag=f"att{p}")
                     for p in range(NPAIR)]

            # ---------- phase 1b: v (natural layout) ----------
            def emit_v(tb):
                ps = psb.tile([128, 1024], F32, name=f"psv{tb}", tag="psb")
                for k in range(KC):
                    nc.tensor.matmul(
                        ps[:, 0:NH * DH],
                        xt_t[k][:, tb * 128:(tb + 1) * 128],
                        wv_t[k][:],
                        start=(k == 0),
                        stop=(k == KC - 1),
                    )
                nc.gpsimd.memset(v_t[tb][:], 1.0)
                nc.vector.tensor_copy(
                    v_t[tb][:].rearrange("p (h e) -> p h e", e=DH + 1)[:, :, 0:DH],
                    ps[:, 0:NH * DH].rearrange("p (h e) -> p h e", e=DH),
                )

            for tb in range(TT):
                emit_v(tb)

            head_order = [1, 0, 3, 2, 5, 4, 7, 6]   # B before A per pair
            dup_q = {}
            dup_k = {}

            def emit_dup_half(h, sh):
                """Duplicate head h's q/k rows (s-columns sh*1024:+1024) to
                both partition halves so two t-tiles' ST matmuls row-pack
                (bases 0 and 64)."""
                pr, half = h // 2, h % 2
                r0, r1 = half * 64, half * 64 + 64
                if h not in dup_q:
                    dup_q[h] = sbdp.tile([128, S], BF16, name=f"qd{h}",
                                         tag="qd")
                    dup_k[h] = sbdp.tile([128, S], BF16, name=f"kd{h}",
                                         tag="kd")
                cols = slice(sh * 1024, (sh + 1) * 1024)
                for dst0 in (0, 64):
                    nc.sync.dma_start(dup_q[h][dst0:dst0 + 64, cols],
                                      qk_t[pr][r0:r1, cols])
                    nc.sync.dma_start(dup_k[h][dst0:dst0 + 64, cols],
                                      qk_t[NPAIR + pr][r0:r1, cols])

            def emit_dup(h):
                emit_dup_half(h, 0)
                emit_dup_half(h, 1)

            # ---------- phase 1a: qT / kT for head pair 0 (serial) ----------
            p1a_pair0 = ((NPAIR, 0), (0, 0), (NPAIR, 1), (0, 1))
            dup_after = {(0, 0): 0, (0, 1): 1}   # dup half after these units
            for nb, scp in p1a_pair0:
                ps = psb.tile([128, 1024], F32, name=f"psq{nb}_{scp}",
                              tag="psb")
                for half1 in range(2):
                    sc = 2 * scp + half1
                    for k in range(KC):
                        nc.tensor.matmul(
                            ps[:, half1 * 512:(half1 + 1) * 512],
                            wqk_t[k][:, nb * 128:(nb + 1) * 128],
                            xt_t[k][:, sc * 512:(sc + 1) * 512],
                            start=(k == 0),
                            stop=(k == KC - 1),
                        )
                nc.vector.tensor_copy(
                    qk_t[nb][:, scp * 1024:(scp + 1) * 1024], ps[:])
                if (nb, scp) in dup_after:
                    emit_dup_half(head_order[0], dup_after[(nb, scp)])

            # ---------- phase 2: attention (s-halved, software-pipelined) --
            # Flat stream over (head, s-half pass, t-tile). ScalarE exp is
            # the bottleneck; TensorE rides ~1 step of ST + lagged PV + one
            # drip-fed phase-1a matmul per step (pairs 1-3 of qkT computed
            # inside the attention stream through a dedicated PSUM bank).
            LAG = 3

            # injected phase-1a work: one unit = one (nb, s-chunk) 8-matmul
            # accumulation + cast, split into single-matmul thunks
            def make_inject_thunks():
                # (nb, sc) units, front-loaded: the dup-DMA prefetch for a
                # pair's heads reads the whole qkT tile, so pair p must be
                # complete ~8 tp-steps before its first head starts
                order = []
                releases = []
                u = 0
                for bi, base in enumerate((1, 2, 3)):
                    qn, kn = base, NPAIR + base
                    for sc in range(SC):
                        for nbx in (qn, kn):
                            order.append((nbx, sc))
                            # pair 1 front-loaded (its dup prefetch reads the
                            # whole tile at step 24); pairs 2/3 stretched to
                            # keep the PE fed through step ~85
                            if bi == 0:
                                releases.append(2 * u)
                            else:
                                releases.append(16 * bi + 4 * (u - 8 * bi))
                            u += 1
                thunks = []
                rel = []
                for u, (nb, sc) in enumerate(order):
                    state = {}

                    def mk_mm(k, nb=nb, sc=sc, state=state):
                        def run():
                            if "ps" not in state:
                                state["ps"] = psc.tile(
                                    [128, 512], F32,
                                    name=f"psi{nb}_{sc}", tag="psc")
                            nc.tensor.matmul(
                                state["ps"][:],
                                wqk_t[k][:, nb * 128:(nb + 1) * 128],
                                xt_t[k][:, sc * 512:(sc + 1) * 512],
                                start=(k == 0),
                                stop=(k == KC - 1),
                            )
                        return run

                    def mk_evac(nb=nb, sc=sc, state=state):
                        def run():
                            nc.vector.tensor_copy(
                                qk_t[nb][:, sc * 512:(sc + 1) * 512],
                                state["ps"][:])
                        return run

                    for k in range(KC):
                        thunks.append(mk_mm(k))
                        rel.append(releases[u] + k)
                    thunks.append(mk_evac())
                    rel.append(releases[u] + KC)
                return thunks, rel

            inject, inj_rel = make_inject_thunks()
            inj_pos = 0

            # HAM warm-keeper: harmless matmuls accumulating into a scratch
            # PSUM tile (flushed to a tiny scratch output so DCE keeps them).
            warm_state = {"n": 0, "ps": None}

            def emit_warm():
                if warm_state["ps"] is None:
                    warm_state["ps"] = psc.tile([128, 512], F32,
                                                name="pswarm", tag="psc")
                nc.tensor.matmul(
                    warm_state["ps"][:, 0:128],
                    wqk_t[0][:, 0:128],
                    xt_t[0][:, 0:128],
                    start=(warm_state["n"] == 0),
                    stop=False,
                    skip_group_check=True,
                )
                warm_state["n"] += 1

            def flush_warm():
                if warm_state["ps"] is None:
                    return
                nc.tensor.matmul(
                    warm_state["ps"][:, 0:128],
                    wqk_t[0][:, 0:128],
                    xt_t[0][:, 0:128],
                    start=False,
                    stop=True,
                    skip_group_check=True,
                )
                wsb = sbo.tile([1, 512], F32, name="warm_sb", tag="warm_sb")
                nc.vector.tensor_copy(wsb[:, 0:128], warm_state["ps"][0:1, 0:128])
                nc.sync.dma_start(warm[:, 0:128], wsb[:, 0:128])
                warm_state["ps"] = None

            exp_state = {"n": 0}
            p_tiles = {}    # (h, pa, tt) -> p tile
            accs = {}       # (h, pa) -> [2 psum accumulators]
            r_cs = {}       # (h, pa) -> [2 r tiles]
            stages = {}     # h -> B-head staging tile

            def retire(hp, pap, tpp):
                """Emit PV for t-tiles 2*tpp, 2*tpp+1; after the last pair
                emit the normalization tail inline."""
                h = hp
                if tpp == 0:
                    accs[(hp, pap)] = [
                        psa.tile([128, 512], F32, name=f"pv{hp}_{pap}_{cc}",
                                 tag="psa")
                        for cc in range(2)]
                acc = accs[(hp, pap)]
                for ttp in (2 * tpp, 2 * tpp + 1):
                    pt = p_tiles.pop((hp, pap, ttp))
                    for cc in range(2):
                        nc.tensor.matmul(
                            acc[cc][0:DH + 1, :],
                            v_t[ttp][:, h * (DH + 1):(h + 1) * (DH + 1)],
                            pt[:, cc * 512:(cc + 1) * 512],
                            start=(ttp == 0),
                            stop=(ttp == TT - 1),
                        )
                if tpp == TT // 2 - 1:
                    pr, half = hp // 2, hp % 2
                    if half == 0:
                        dst = att_t[pr]
                    else:
                        if pap == 0:
                            stages[hp] = sbst.tile([64, S], BF16,
                                                   name=f"bst{hp}",
                                                   tag="bstage")
                        dst = stages[hp]
                    for cc in range(2):
                        cg = pap * 2 + cc
                        r = sbr.tile([128, 512], F32R,
                                     name=f"r{hp}_{pap}_{cc}", tag="r")
                        nc.vector.tensor_copy(r[64:65, :], acc[cc][64:65, :])
                        rb = psa.tile([128, 512], F32,
                                      name=f"rb{hp}_{pap}_{cc}", tag="psa")
                        nc.tensor.matmul(
                            rb[0:64, :],
                            ones_r[64:65, 0:64],
                            r[64:65, :],
                            start=True,
                            stop=True,
                        )
                        rinv = sbrv.tile([64, 512], F32,
                                         name=f"rinv{hp}_{pap}_{cc}",
                                         tag="rinv")
                        nc.vector.reciprocal_approx_fast(rinv[:], rb[0:64, :])
                        nc.vector.tensor_mul(
                            dst[0:64, cg * 512:(cg + 1) * 512],
                            acc[cc][0:64, :],
                            rinv[:],
                        )
                    if half == 1 and pap == 1:
                        nc.sync.dma_start(att_t[pr][64:128, :],
                                          stages[hp][0:64, :])

            seq = [(h, pa, tp)
                   for h in head_order for pa in range(2)
                   for tp in range(TT // 2)]
            for i, (h, pa, tp) in enumerate(seq):
                qd, kd = dup_q[h], dup_k[h]
                tt0, tt1 = 2 * tp, 2 * tp + 1

                # emit PV/injection BEFORE the ST quad: the scheduler pops
                # ready work in priority (program) order, so nothing can
                # interleave into the row-packed A/B matmul pairs
                if pa == 1 and tp == 4:
                    # prefetch dup tiles for the next head (12 steps ahead;
                    # late enough that injected qkT writes precede the read)
                    hi = head_order.index(h)
                    if hi + 1 < len(head_order):
                        emit_dup(head_order[hi + 1])
                if i >= LAG:
                    retire(*seq[i - LAG])
                ran = 0
                while (inj_pos < len(inject) and inj_rel[inj_pos] <= i
                       and ran < 4):
                    inject[inj_pos]()
                    inj_pos += 1
                    ran += 1
                if ran == 0:
                    emit_warm()

                ptA = sbp.tile([128, 1024], BF16, name=f"p{h}_{pa}_{tt0}",
                               tag="p")
                ptB = sbp.tile([128, 1024], BF16, name=f"p{h}_{pa}_{tt1}",
                               tag="p")
                p_tiles[(h, pa, tt0)] = ptA
                p_tiles[(h, pa, tt1)] = ptB
                psA = psb.tile([128, 1024], F32, name=f"pstA{h}_{pa}_{tp}",
                               tag="psb")
                psB = psb.tile([128, 1024], F32, name=f"pstB{h}_{pa}_{tp}",
                               tag="psb")
                for j in range(2):
                    nc.tensor.matmul(
                        psA[:, j * 512:(j + 1) * 512],
                        kd[0:64, tt0 * 128:(tt0 + 1) * 128],
                        qd[0:64, pa * 1024 + j * 512:pa * 1024 + (j + 1) * 512],
                        start=True,
                        stop=True,
                    )
                    nc.tensor.matmul(
                        psB[:, j * 512:(j + 1) * 512],
                        kd[64:128, tt1 * 128:(tt1 + 1) * 128],
                        qd[64:128, pa * 1024 + j * 512:pa * 1024 + (j + 1) * 512],
                        start=True,
                        stop=True,
                    )
                for dst, src in ((ptA, psA), (ptB, psB)):
                    xi = exp_state["n"]
                    exp_state["n"] += 1
                    if DVE_EXP_RATIO and xi % DVE_EXP_RATIO == 2:
                        # offload this half-tile's exp to the DVE:
                        # P(x) cubic then P^64 (see module header)
                        tmp = sbxp.tile([128, 1024], F32, name=f"xp{xi}",
                                        tag="xp")
                        nc.vector._custom_dve(
                            EXP_P3_ANT, out=tmp[:], in0=src[:],
                            in1=c3col[:], s0=_EXP_COEF[0], s1=_EXP_COEF[1],
                            imm2=_EXP_COEF[2])
                        nc.vector._custom_dve(POW64_ANT, out=dst[:],
                                              in0=tmp[:])
                    else:
                        nc.scalar.activation(
                            dst[:], src[:], mybir.ActivationFunctionType.Exp,
                            scale=0.125)
            while inj_pos < len(inject):
                inject[inj_pos]()
                inj_pos += 1
            for i in range(len(seq) - LAG, len(seq)):
                retire(*seq[i])
                for _ in range(6):
                    emit_warm()
            for _ in range(12):
                emit_warm()
            flush_warm()

            # ---------- phase 3: output projection ----------
            # attention pools are idle now; rotate psum groups across all
            # three pools for deeper pipelining
            for sb_i in range(TT):
                for e in range(2):
                    sel = (2 * sb_i + e) % 4
                    if sel < 2:
                        ps = psa.tile([128, 512], F32, name=f"po{sb_i}_{e}",
                                      tag="psa")
                    elif sel == 2:
                        psw = psb.tile([128, 1024], F32, name=f"po{sb_i}_{e}",
                                       tag="psb")
                        ps = psw[:, 0:512]
                    else:
                        ps = psc.tile([128, 512], F32, name=f"po{sb_i}_{e}",
                                      tag="psc")
                    for pr in range(NPAIR):
                        nc.tensor.matmul(
                            ps[:],
                            att_t[pr][:, sb_i * 128:(sb_i + 1) * 128],
                            wout_t[pr][:, e * 512:(e + 1) * 512],
                            start=(pr == 0),
                            stop=(pr == NPAIR - 1),
                        )
                    o = sbo.tile([128, 512], F32, name=f"o{sb_i}_{e}", tag="o")
                    nc.vector.tensor_copy(o[:], ps[:])
                    # output DMA alternates sync/scalar HW queues (ScalarE is
                    # idle once phase 2's activations are done)
                    oeng = nc.sync if (2 * sb_i + e) % 2 == 0 else nc.scalar
                    oeng.dma_start(
                        out_p[sb_i * 128:(sb_i + 1) * 128,
                              e * 512:(e + 1) * 512], o[:])

    nc.compile()
    return nc


def get_nc():
    global _CACHED_NC
    if _CACHED_NC is None:
        _CACHED_NC = build_nc()
    return _CACHED_NC


def make_in_maps(x, w_qkv, w_out):
    bf = ml_dtypes.bfloat16
    in_maps = []
    for c in range(N_CORES):
        b = c // 2
        h0 = (c % 2) * NH
        q_cols = w_qkv[:, h0 * DH:(h0 + NH) * DH]
        k_cols = w_qkv[:, D + h0 * DH:D + (h0 + NH) * DH]
        v_cols = w_qkv[:, 2 * D + h0 * DH:2 * D + (h0 + NH) * DH]
        in_maps.append({
            "xt": np.ascontiguousarray(x[b].T).astype(bf),
            "wqk": np.concatenate([q_cols, k_cols], axis=1).astype(bf),
            "wv": np.ascontiguousarray(v_cols).astype(bf),
            "wout": np.ascontiguousarray(
                w_out[h0 * DH:(h0 + NH) * DH, :]).astype(bf),
        })
    return in_maps


def run(x, w_qkv, w_out, trace=False, trace_cores=None):
    nc = get_nc()
    in_maps = make_in_maps(x, w_qkv, w_out)
    res = bass_utils.run_bass_kernel_spmd(
        nc, in_maps, core_ids=list(range(N_CORES)),
        trace=trace, trace_cores=trace_cores,
    )
    partials = [res.results[c]["out_p"] for c in range(N_CORES)]
    out = np.stack([partials[2 * b] + partials[2 * b + 1] for b in range(4)])
    return out.astype(np.float32), res


def kernel(x, w_qkv, w_out):
    out, _ = run(np.asarray(x), np.asarray(w_qkv), np.asarray(w_out))
    return out


if __name__ == "__main__":
    import tempfile
    nc = build_nc()
    with tempfile.TemporaryDirectory() as td:
        neff = bass_utils.compile_bass_kernel(nc, td)
        print("LOCAL COMPILE OK:", neff)



# revision 29
# speedup vs baseline: 1.0651x; 1.0103x over previous
"""Multi-head attention block (nn_AttentionBlock) on 8 Trainium2 NeuronCores.

Reference computation (fp32):
    qkv = x @ w_qkv;  q,k,v = split(qkv)
    per head: att = softmax(q @ k.T / 8) @ v
    out = concat_heads(att) @ w_out
Shapes: x [4, 2048, 1024], w_qkv [1024, 3072], w_out [1024, 1024], 16 heads.

Sharding: batch x head-half. Core c handles batch b = c//2 and heads
h0 = (c%2)*8 .. h0+8. Each core computes its 8 heads' attention and a partial
output projection (row-shard of w_out); host sums the two partials per batch.

Per-core kernel (all heavy matmuls bf16 with fp32 PSUM accumulation):
  P1: qkT[n, s] = w_qk.T @ x.T (weights stationary, xT moving) -> q/k heads
      transposed in SBUF; v[t, d] = x @ w_v in natural layout with an
      appended ones column per head (v_aug).
  P2 per head pair (2 heads share a 128-partition tile, K=64 row-packed
      matmuls at base partitions 0/64):
      ST[t, s] = exp((k_t . q_s)/8)  via TensorE + ScalarE exp (psum->sbuf
      bf16), no max subtraction (logits are O(5), exp is safe in fp32).
      PV: att_aug[d|1, s] = v_aug.T @ p accumulated over 16 t-chunks; row 64
      is r[s] = sum_t p. rinv broadcast via K=1 ones matmul + reciprocal;
      normalize att rows with a tensor-tensor multiply.
  P3: partial[s, e] = att.T @ w_out accumulated over the 4 head pairs.
"""

import sys

sys.path.insert(0, "/opt/trn_rl_repo")

import numpy as np
import ml_dtypes

import concourse.mybir as mybir
import concourse.tile as tile
from concourse import bacc
from concourse import bass_utils
from concourse import dve_ops as _dve_ops
from concourse.dve_spec import (
    Spec, Src0, C0, C1, C2, C3, sq, lower, _spill_c3_to_src1, _has_src1,
)
from concourse.dve_uop import DveOpSpec

F32 = mybir.dt.float32
F32R = mybir.dt.float32r
BF16 = mybir.dt.bfloat16

# ---------------------------------------------------------------------------
# Custom DVE exp: exp(x/8) = P(x)^64 with P a cubic ~ e^(x/512).
# Pass 1 (EXP_P3_ANT): Horner cubic, coefficients pre-scaled by s=1/512.
# Pass 2 (POW64_ANT): six chained squares.
# Max rel err 5.5e-5 over x in [-56, 56] (logits here are O(6) * 8).
# Registered into dve_ops at import so the per-NEFF DVE table includes them.
_EXP_S = 0.125 / 64.0
_EXP_COEF = (
    0.9999992332857001,                      # c0
    0.9999996170667287 * _EXP_S,             # c1*s
    0.5005056061615765 * _EXP_S ** 2,        # c2*s^2
    0.16679299496749722 * _EXP_S ** 3,       # c3*s^3  (via C3 -> in1 latch)
)


def _np_ref_exp_p3(in0, in1, s0, s1, imm2):
    c3 = np.asarray(in1, dtype=np.float32).reshape(-1)[0]
    x = np.asarray(in0, dtype=np.float32)
    return (((c3 * x + imm2) * x + s1) * x + s0).astype(np.float32)


def _np_ref_pow64(in0, in1, s0, s1, imm2):
    x = np.asarray(in0, dtype=np.float32)
    for _ in range(6):
        x = x * x
    return x.astype(np.float32)


def _register_exp_dve_ops():
    if "EXP_P3_ANT" in _dve_ops._SUB_OPCODE_FOR_NAME:
        return ([op for op in _dve_ops.OPS if op.name == "EXP_P3_ANT"][0],
                [op for op in _dve_ops.OPS if op.name == "POW64_ANT"][0])
    specs = [
        ("EXP_P3_ANT",
         Spec(body=_spill_c3_to_src1(((C3 * Src0 + C2) * Src0 + C1) * Src0
                                     + C0),
              reference=_np_ref_exp_p3)),
        ("POW64_ANT",
         Spec(body=sq(sq(sq(sq(sq(sq(Src0)))))), reference=_np_ref_pow64)),
    ]
    made = []
    for name, spec in specs:
        row = _dve_ops._CUSTOM_DVE_ROW_BASE + len(_dve_ops.OPS)
        shas = {}
        for ver in ("v3", "v4"):
            dos = DveOpSpec(name=name, opcode=row, uops=lower(spec, ver=ver),
                            rd1_en=_has_src1(spec))
            shas[ver] = dos.sha(ver)
        op = _dve_ops.DveOp(name, spec, subdim=False, uops_sha=shas)
        _dve_ops.OPS.append(op)
        _dve_ops._SUB_OPCODE_FOR_NAME[name] = row
        _dve_ops.CUSTOM_DVE_SPECS[name] = spec
        made.append(op)
    return tuple(made)


EXP_P3_ANT, POW64_ANT = _register_exp_dve_ops()

S = 2048          # sequence length
D = 1024          # embed dim
DH = 64           # head dim
NH = 8            # heads per core
NPAIR = 4         # head pairs per core
KC = D // 128     # contraction chunks for phase 1
SC = S // 512     # 512-wide free chunks of the sequence
TT = S // 128     # 128-row t tiles of the sequence
N_CORES = 8
# 1 of every N exp half-tiles runs on the DVE; 0 disables (the custom-DVE
# exp path crashes NRT on this runtime build — kept for reference)
DVE_EXP_RATIO = 0

_CACHED_NC = None


def build_nc():
    nc = bacc.Bacc("TRN2", target_bir_lowering=False, debug=False,
                   num_devices=N_CORES)

    xt = nc.dram_tensor("xt", [D, S], BF16, kind="ExternalInput").ap()
    wqk = nc.dram_tensor("wqk", [D, D], BF16, kind="ExternalInput").ap()
    wv = nc.dram_tensor("wv", [D, NH * DH], BF16, kind="ExternalInput").ap()
    wout = nc.dram_tensor("wout", [NH * DH, D], BF16, kind="ExternalInput").ap()
    out_p = nc.dram_tensor("out_p", [S, D], F32, kind="ExternalOutput").ap()
    warm = nc.dram_tensor("warm", [1, 512], F32, kind="ExternalOutput").ap()

    with tile.TileContext(nc) as tc:
        with (
            tc.tile_pool(name="sbc", bufs=1) as sbc,        # constants
            tc.tile_pool(name="sbin", bufs=1) as sbin,      # phase-1 inputs
            tc.tile_pool(name="sbqk", bufs=1) as sbqk,      # qkT persistent
            tc.tile_pool(name="sbv", bufs=1) as sbv,        # v_aug persistent
            tc.tile_pool(name="sbp", bufs=10) as sbp,        # exp(ST) tiles
            tc.tile_pool(name="sbr", bufs=4) as sbr,        # r chunks (f32r)
            tc.tile_pool(name="sbrv", bufs=2) as sbrv,      # rinv chunks
            tc.tile_pool(name="sbst", bufs=2) as sbst,      # B-head att stage
            tc.tile_pool(name="sbdp", bufs=2) as sbdp,      # dup q/k scratch
            tc.tile_pool(name="sbat", bufs=1) as sbat,      # att pairs
            tc.tile_pool(name="sbo", bufs=4) as sbo,        # out evac
            tc.tile_pool(name="sbxp", bufs=1) as sbxp,      # DVE-exp scratch
            tc.tile_pool(name="psb", bufs=2, space="PSUM") as psb,   # [128,1024]
            tc.tile_pool(name="psa", bufs=3, space="PSUM") as psa,   # [128,512]
            tc.tile_pool(name="psc", bufs=1, space="PSUM") as psc,   # [128,512]
        ):
            # ---------- constants ----------
            ones_f = sbc.tile([128, 128], F32, name="ones_f")
            nc.gpsimd.memset(ones_f[:], 1.0)
            ones_r = sbc.tile([128, 128], F32R, name="ones_r")
            nc.vector.tensor_copy(ones_r[:], ones_f[:])
            c3col = sbc.tile([128, 1], F32, name="c3col")
            nc.gpsimd.memset(c3col[:], _EXP_COEF[3])

            # ---------- input DMA ----------
            xt_t = [sbin.tile([128, S], BF16, name=f"xt{k}", tag=f"xt{k}")
                    for k in range(KC)]
            wqk_t = [sbin.tile([128, D], BF16, name=f"wqk{k}", tag=f"wqk{k}")
                     for k in range(KC)]
            wv_t = [sbin.tile([128, NH * DH], BF16, name=f"wv{k}", tag=f"wv{k}")
                    for k in range(KC)]
            wout_t = [sbin.tile([128, D], BF16, name=f"wout{p}", tag=f"wout{p}")
                      for p in range(NPAIR)]
            # wv first (phase 1b needs it in full), xt s-chunk-major (phase 1b
            # consumes s-chunks in order), wqk early (phase 1a gates phase 2),
            # wout last. Spread across three DMA queues (sync HW, vector SW,
            # gpsimd SW) so the load isn't serialized on one ring.
            for k in range(KC):
                nc.gpsimd.dma_start(wv_t[k][:], wv[k * 128:(k + 1) * 128, :])
            for sc in range(SC):
                for k in range(KC):
                    nc.sync.dma_start(
                        xt_t[k][:, sc * 512:(sc + 1) * 512],
                        xt[k * 128:(k + 1) * 128, sc * 512:(sc + 1) * 512])
            for k in range(KC):
                nc.sync.dma_start(wqk_t[k][:], wqk[k * 128:(k + 1) * 128, :])
            for p in range(NPAIR):
                nc.sync.dma_start(wout_t[p][:], wout[p * 128:(p + 1) * 128, :])

            # ---------- persistent intermediates ----------
            # qkT rows: tiles 0..3 = qT head pairs, 4..7 = kT head pairs
            qk_t = [sbqk.tile([128, S], BF16, name=f"qk{n}", tag=f"qk{n}")
                    for n in range(2 * NPAIR)]
            # v_aug: [t, 8*(64+1)]; per head 64 v columns then a ones column
            v_t = [sbv.tile([128, NH * (DH + 1)], BF16, name=f"v{t}", tag=f"v{t}")
                   for t in range(TT)]
            att_t = [sbat.tile([128, S], BF16, name=f"att{p}", tag=f"att{p}")
                     for p in range(NPAIR)]

            # ---------- phase 1b: v (natural layout) ----------
            def emit_v(tb):
                ps = psb.tile([128, 1024], F32, name=f"psv{tb}", tag="psb")
                for k in range(KC):
                    nc.tensor.matmul(
                        ps[:, 0:NH * DH],
                        xt_t[k][:, tb * 128:(tb + 1) * 128],
                        wv_t[k][:],
                        start=(k == 0),
                        stop=(k == KC - 1),
                    )
                nc.gpsimd.memset(v_t[tb][:], 1.0)
                nc.vector.tensor_copy(
                    v_t[tb][:].rearrange("p (h e) -> p h e", e=DH + 1)[:, :, 0:DH],
                    ps[:, 0:NH * DH].rearrange("p (h e) -> p h e", e=DH),
                )

            for tb in range(TT):
                emit_v(tb)

            head_order = [1, 0, 3, 2, 5, 4, 7, 6]   # B before A per pair
            dup_q = {}
            dup_k = {}

            def emit_dup_half(h, sh):
                """Duplicate head h's q/k rows (s-columns sh*1024:+1024) to
                both partition halves so two t-tiles' ST matmuls row-pack
                (bases 0 and 64)."""
                pr, half = h // 2, h % 2
                r0, r1 = half * 64, half * 64 + 64
                if h not in dup_q:
                    dup_q[h] = sbdp.tile([128, S], BF16, name=f"qd{h}",
                                         tag="qd")
                    dup_k[h] = sbdp.tile([128, S], BF16, name=f"kd{h}",
                                         tag="kd")
                cols = slice(sh * 1024, (sh + 1) * 1024)
                for dst0 in (0, 64):
                    nc.sync.dma_start(dup_q[h][dst0:dst0 + 64, cols],
                                      qk_t[pr][r0:r1, cols])
                    nc.sync.dma_start(dup_k[h][dst0:dst0 + 64, cols],
                                      qk_t[NPAIR + pr][r0:r1, cols])

            def emit_dup(h):
                emit_dup_half(h, 0)
                emit_dup_half(h, 1)

            # ---------- phase 1a: qT / kT for head pair 0 (serial) ----------
            p1a_pair0 = ((NPAIR, 0), (0, 0), (NPAIR, 1), (0, 1))
            dup_after = {(0, 0): 0, (0, 1): 1}   # dup half after these units
            for nb, scp in p1a_pair0:
                ps = psb.tile([128, 1024], F32, name=f"psq{nb}_{scp}",
                              tag="psb")
                for half1 in range(2):
                    sc = 2 * scp + half1
                    for k in range(KC):
                        nc.tensor.matmul(
                            ps[:, half1 * 512:(half1 + 1) * 512],
                            wqk_t[k][:, nb * 128:(nb + 1) * 128],
                            xt_t[k][:, sc * 512:(sc + 1) * 512],
                            start=(k == 0),
                            stop=(k == KC - 1),
                        )
                nc.vector.tensor_copy(
                    qk_t[nb][:, scp * 1024:(scp + 1) * 1024], ps[:])
                if (nb, scp) in dup_after:
                    emit_dup_half(head_order[0], dup_after[(nb, scp)])

            # ---------- phase 2: attention (s-halved, software-pipelined) --
            # Flat stream over (head, s-half pass, t-tile). ScalarE exp is
            # the bottleneck; TensorE rides ~1 step of ST + lagged PV + one
            # drip-fed phase-1a matmul per step (pairs 1-3 of qkT computed
            # inside the attention stream through a dedicated PSUM bank).
            LAG = 3

            # injected phase-1a work: one unit = one (nb, s-chunk) 8-matmul
            # accumulation + cast, split into single-matmul thunks
            def make_inject_thunks():
                # (nb, sc) units, front-loaded: the dup-DMA prefetch for a
                # pair's heads reads the whole qkT tile, so pair p must be
                # complete ~8 tp-steps before its first head starts
                order = []
                releases = []
                u = 0
                for bi, base in enumerate((1, 2, 3)):
                    qn, kn = base, NPAIR + base
                    for sc in range(SC):
                        for nbx in (qn, kn):
                            order.append((nbx, sc))
                            # pair 1 front-loaded (its dup prefetch reads the
                            # whole tile at step 24); pairs 2/3 stretched to
                            # keep the PE fed through step ~85
                            if bi == 0:
                                releases.append(2 * u)
                            else:
                                releases.append(16 * bi + 4 * (u - 8 * bi))
                            u += 1
                thunks = []
                rel = []
                for u, (nb, sc) in enumerate(order):
                    state = {}

                    def mk_mm(k, nb=nb, sc=sc, state=state):
                        def run():
                            if "ps" not in state:
                                state["ps"] = psc.tile(
                                    [128, 512], F32,
                                    name=f"psi{nb}_{sc}", tag="psc")
                            nc.tensor.matmul(
                                state["ps"][:],
                                wqk_t[k][:, nb * 128:(nb + 1) * 128],
                                xt_t[k][:, sc * 512:(sc + 1) * 512],
                                start=(k == 0),
                                stop=(k == KC - 1),
                            )
                        return run

                    def mk_evac(nb=nb, sc=sc, state=state):
                        def run():
                            nc.vector.tensor_copy(
                                qk_t[nb][:, sc * 512:(sc + 1) * 512],
                                state["ps"][:])
                        return run

                    for k in range(KC):
                        thunks.append(mk_mm(k))
                        rel.append(releases[u] + k)
                    thunks.append(mk_evac())
                    rel.append(releases[u] + KC)
                return thunks, rel

            inject, inj_rel = make_inject_thunks()
            inj_pos = 0

            # HAM warm-keeper: harmless matmuls accumulating into a scratch
            # PSUM tile (flushed to a tiny scratch output so DCE keeps them).
            warm_state = {"n": 0, "ps": None}

            def emit_warm():
                if warm_state["ps"] is None:
                    warm_state["ps"] = psc.tile([128, 512], F32,
                                                name="pswarm", tag="psc")
                nc.tensor.matmul(
                    warm_state["ps"][:],
                    wqk_t[0][:, 0:128],
                    xt_t[0][:, 0:512],
                    start=(warm_state["n"] == 0),
                    stop=False,
                    skip_group_check=True,
                )
                warm_state["n"] += 1

            def flush_warm():
                if warm_state["ps"] is None:
                    return
                nc.tensor.matmul(
                    warm_state["ps"][:],
                    wqk_t[0][:, 0:128],
                    xt_t[0][:, 0:512],
                    start=False,
                    stop=True,
                    skip_group_check=True,
                )
                wsb = sbo.tile([1, 512], F32, name="warm_sb", tag="warm_sb")
                nc.vector.tensor_copy(wsb[:], warm_state["ps"][0:1, :])
                nc.sync.dma_start(warm[:, :], wsb[:])
                warm_state["ps"] = None

            exp_state = {"n": 0}
            p_tiles = {}    # (h, pa, tt) -> p tile
            accs = {}       # (h, pa) -> [2 psum accumulators]
            r_cs = {}       # (h, pa) -> [2 r tiles]
            stages = {}     # h -> B-head staging tile

            def retire(hp, pap, tpp):
                """Emit PV for t-tiles 2*tpp, 2*tpp+1; after the last pair
                emit the normalization tail inline."""
                h = hp
                if tpp == 0:
                    accs[(hp, pap)] = [
                        psa.tile([128, 512], F32, name=f"pv{hp}_{pap}_{cc}",
                                 tag="psa")
                        for cc in range(2)]
                acc = accs[(hp, pap)]
                for ttp in (2 * tpp, 2 * tpp + 1):
                    pt = p_tiles.pop((hp, pap, ttp))
                    for cc in range(2):
                        nc.tensor.matmul(
                            acc[cc][0:DH + 1, :],
                            v_t[ttp][:, h * (DH + 1):(h + 1) * (DH + 1)],
                            pt[:, cc * 512:(cc + 1) * 512],
                            start=(ttp == 0),
                            stop=(ttp == TT - 1),
                        )
                if tpp == TT // 2 - 1:
                    pr, half = hp // 2, hp % 2
                    if half == 0:
                        dst = att_t[pr]
                    else:
                        if pap == 0:
                            stages[hp] = sbst.tile([64, S], BF16,
                                                   name=f"bst{hp}",
                                                   tag="bstage")
                        dst = stages[hp]
                    for cc in range(2):
                        cg = pap * 2 + cc
                        r = sbr.tile([128, 512], F32R,
                                     name=f"r{hp}_{pap}_{cc}", tag="r")
                        nc.vector.tensor_copy(r[64:65, :], acc[cc][64:65, :])
                        rb = psa.tile([128, 512], F32,
                                      name=f"rb{hp}_{pap}_{cc}", tag="psa")
                        nc.tensor.matmul(
                            rb[0:64, :],
                            ones_r[64:65, 0:64],
                            r[64:65, :],
                            start=True,
                            stop=True,
                        )
                        rinv = sbrv.tile([64, 512], F32,
                                         name=f"rinv{hp}_{pap}_{cc}",
                                         tag="rinv")
                        nc.vector.reciprocal_approx_fast(rinv[:], rb[0:64, :])
                        nc.vector.tensor_mul(
                            dst[0:64, cg * 512:(cg + 1) * 512],
                            acc[cc][0:64, :],
                            rinv[:],
                        )
                    if half == 1 and pap == 1:
                        nc.sync.dma_start(att_t[pr][64:128, :],
                                          stages[hp][0:64, :])

            seq = [(h, pa, tp)
                   for h in head_order for pa in range(2)
                   for tp in range(TT // 2)]
            for i, (h, pa, tp) in enumerate(seq):
                qd, kd = dup_q[h], dup_k[h]
                tt0, tt1 = 2 * tp, 2 * tp + 1

                # emit PV/injection BEFORE the ST quad: the scheduler pops
                # ready work in priority (program) order, so nothing can
                # interleave into the row-packed A/B matmul pairs
                if pa == 1 and tp == 4:
                    # prefetch dup tiles for the next head (12 steps ahead;
                    # late enough that injected qkT writes precede the read)
                    hi = head_order.index(h)
                    if hi + 1 < len(head_order):
                        emit_dup(head_order[hi + 1])
                if i >= LAG:
                    retire(*seq[i - LAG])
                ran = 0
                while (inj_pos < len(inject) and inj_rel[inj_pos] <= i
                       and ran < 4):
                    inject[inj_pos]()
                    inj_pos += 1
                    ran += 1
                if ran == 0:
                    emit_warm()

                ptA = sbp.tile([128, 1024], BF16, name=f"p{h}_{pa}_{tt0}",
                               tag="p")
                ptB = sbp.tile([128, 1024], BF16, name=f"p{h}_{pa}_{tt1}",
                               tag="p")
                p_tiles[(h, pa, tt0)] = ptA
                p_tiles[(h, pa, tt1)] = ptB
                psA = psb.tile([128, 1024], F32, name=f"pstA{h}_{pa}_{tp}",
                               tag="psb")
                psB = psb.tile([128, 1024], F32, name=f"pstB{h}_{pa}_{tp}",
                               tag="psb")
                for j in range(2):
                    nc.tensor.matmul(
                        psA[:, j * 512:(j + 1) * 512],
                        kd[0:64, tt0 * 128:(tt0 + 1) * 128],
                        qd[0:64, pa * 1024 + j * 512:pa * 1024 + (j + 1) * 512],
                        start=True,
                        stop=True,
                    )
                    nc.tensor.matmul(
                        psB[:, j * 512:(j + 1) * 512],
                        kd[64:128, tt1 * 128:(tt1 + 1) * 128],
                        qd[64:128, pa * 1024 + j * 512:pa * 1024 + (j + 1) * 512],
                        start=True,
                        stop=True,
                    )
                for dst, src in ((ptA, psA), (ptB, psB)):
                    xi = exp_state["n"]
                    exp_state["n"] += 1
                    if DVE_EXP_RATIO and xi % DVE_EXP_RATIO == 2:
                        # offload this half-tile's exp to the DVE:
                        # P(x) cubic then P^64 (see module header)
                        tmp = sbxp.tile([128, 1024], F32, name=f"xp{xi}",
                                        tag="xp")
                        nc.vector._custom_dve(
                            EXP_P3_ANT, out=tmp[:], in0=src[:],
                            in1=c3col[:], s0=_EXP_COEF[0], s1=_EXP_COEF[1],
                            imm2=_EXP_COEF[2])
                        nc.vector._custom_dve(POW64_ANT, out=dst[:],
                                              in0=tmp[:])
                    else:
                        nc.scalar.activation(
                            dst[:], src[:], mybir.ActivationFunctionType.Exp,
                            scale=0.125)
            while inj_pos < len(inject):
                inject[inj_pos]()
                inj_pos += 1
            for i in range(len(seq) - LAG, len(seq)):
                retire(*seq[i])
                for _ in range(6):
                    emit_warm()
            for _ in range(12):
                emit_warm()
            flush_warm()

            # ---------- phase 3: output projection ----------
            # attention pools are idle now; rotate psum groups across all
            # three pools for deeper pipelining
            for sb_i in range(TT):
                for e in range(2):
                    sel = (2 * sb_i + e) % 4
                    if sel < 2:
                        ps = psa.tile([128, 512], F32, name=f"po{sb_i}_{e}",
                                      tag="psa")
                    elif sel == 2:
                        psw = psb.tile([128, 1024], F32, name=f"po{sb_i}_{e}",
                                       tag="psb")
                        ps = psw[:, 0:512]
                    else:
                        ps = psc.tile([128, 512], F32, name=f"po{sb_i}_{e}",
                                      tag="psc")
                    for pr in range(NPAIR):
                        nc.tensor.matmul(
                            ps[:],
                            att_t[pr][:, sb_i * 128:(sb_i + 1) * 128],
                            wout_t[pr][:, e * 512:(e + 1) * 512],
                            start=(pr == 0),
                            stop=(pr == NPAIR - 1),
                        )
                    o = sbo.tile([128, 512], F32, name=f"o{sb_i}_{e}", tag="o")
                    nc.vector.tensor_copy(o[:], ps[:])
                    # output DMA alternates sync/scalar HW queues (ScalarE is
                    # idle once phase 2's activations are done)
                    oeng = nc.sync if (2 * sb_i + e) % 2 == 0 else nc.scalar
                    oeng.dma_start(
                        out_p[sb_i * 128:(sb_i + 1) * 128,
                              e * 512:(e + 1) * 512], o[:])

    nc.compile()
    return nc


def get_nc():
    global _CACHED_NC
    if _CACHED_NC is None:
        _CACHED_NC = build_nc()
    return _CACHED_NC


def make_in_maps(x, w_qkv, w_out):
    bf = ml_dtypes.bfloat16
    in_maps = []
    for c in range(N_CORES):
        b = c // 2
        h0 = (c % 2) * NH
        q_cols = w_qkv[:, h0 * DH:(h0 + NH) * DH]
        k_cols = w_qkv[:, D + h0 * DH:D + (h0 + NH) * DH]
        v_cols = w_qkv[:, 2 * D + h0 * DH:2 * D + (h0 + NH) * DH]
        in_maps.append({
            "xt": np.ascontiguousarray(x[b].T).astype(bf),
            "wqk": np.concatenate([q_cols, k_cols], axis=1).astype(bf),
            "wv": np.ascontiguousarray(v_cols).astype(bf),
            "wout": np.ascontiguousarray(
                w_out[h0 * DH:(h0 + NH) * DH, :]).astype(bf),
        })
    return in_maps


def run(x, w_qkv, w_out, trace=False, trace_cores=None):
    nc = get_nc()
    in_maps = make_in_maps(x, w_qkv, w_out)
    res = bass_utils.run_bass_kernel_spmd(
        nc, in_maps, core_ids=list(range(N_CORES)),
        trace=trace, trace_cores=trace_cores,
    )
    partials = [res.results[c]["out_p"] for c in range(N_CORES)]
    out = np.stack([partials[2 * b] + partials[2 * b + 1] for b in range(4)])
    return out.astype(np.float32), res


def kernel(x, w_qkv, w_out):
    out, _ = run(np.asarray(x), np.asarray(w_qkv), np.asarray(w_out))
    return out


if __name__ == "__main__":
    import tempfile
    nc = build_nc()
    with tempfile.TemporaryDirectory() as td:
        neff = bass_utils.compile_bass_kernel(nc, td)
        print("LOCAL COMPILE OK:", neff)



# revision 30
# speedup vs baseline: 1.0693x; 1.0039x over previous
"""Multi-head attention block (nn_AttentionBlock) on 8 Trainium2 NeuronCores.

Reference computation (fp32):
    qkv = x @ w_qkv;  q,k,v = split(qkv)
    per head: att = softmax(q @ k.T / 8) @ v
    out = concat_heads(att) @ w_out
Shapes: x [4, 2048, 1024], w_qkv [1024, 3072], w_out [1024, 1024], 16 heads.

Sharding: batch x head-half. Core c handles batch b = c//2 and heads
h0 = (c%2)*8 .. h0+8. Each core computes its 8 heads' attention and a partial
output projection (row-shard of w_out); host sums the two partials per batch.

Per-core kernel (all heavy matmuls bf16 with fp32 PSUM accumulation):
  P1: qkT[n, s] = w_qk.T @ x.T (weights stationary, xT moving) -> q/k heads
      transposed in SBUF; v[t, d] = x @ w_v in natural layout with an
      appended ones column per head (v_aug).
  P2 per head pair (2 heads share a 128-partition tile, K=64 row-packed
      matmuls at base partitions 0/64):
      ST[t, s] = exp((k_t . q_s)/8)  via TensorE + ScalarE exp (psum->sbuf
      bf16), no max subtraction (logits are O(5), exp is safe in fp32).
      PV: att_aug[d|1, s] = v_aug.T @ p accumulated over 16 t-chunks; row 64
      is r[s] = sum_t p. rinv broadcast via K=1 ones matmul + reciprocal;
      normalize att rows with a tensor-tensor multiply.
  P3: partial[s, e] = att.T @ w_out accumulated over the 4 head pairs.
"""

import sys

sys.path.insert(0, "/opt/trn_rl_repo")

import numpy as np
import ml_dtypes

import concourse.mybir as mybir
import concourse.tile as tile
from concourse import bacc
from concourse import bass_utils

F32 = mybir.dt.float32
F32R = mybir.dt.float32r
BF16 = mybir.dt.bfloat16

S = 2048          # sequence length
D = 1024          # embed dim
DH = 64           # head dim
NH = 8            # heads per core
NPAIR = 4         # head pairs per core
KC = D // 128     # contraction chunks for phase 1
SC = S // 512     # 512-wide free chunks of the sequence
TT = S // 128     # 128-row t tiles of the sequence
N_CORES = 8

_CACHED_NC = None


def build_nc():
    nc = bacc.Bacc("TRN2", target_bir_lowering=False, debug=False,
                   num_devices=N_CORES)

    xt = nc.dram_tensor("xt", [D, S], BF16, kind="ExternalInput").ap()
    wqk = nc.dram_tensor("wqk", [D, D], BF16, kind="ExternalInput").ap()
    wv = nc.dram_tensor("wv", [D, NH * DH], BF16, kind="ExternalInput").ap()
    wout = nc.dram_tensor("wout", [NH * DH, D], BF16, kind="ExternalInput").ap()
    out_p = nc.dram_tensor("out_p", [S, D], F32, kind="ExternalOutput").ap()
    warm = nc.dram_tensor("warm", [1, 512], F32, kind="ExternalOutput").ap()

    with tile.TileContext(nc) as tc:
        with (
            tc.tile_pool(name="sbc", bufs=1) as sbc,        # constants
            tc.tile_pool(name="sbin", bufs=1) as sbin,      # phase-1 inputs
            tc.tile_pool(name="sbqk", bufs=1) as sbqk,      # qkT persistent
            tc.tile_pool(name="sbv", bufs=1) as sbv,        # v_aug persistent
            tc.tile_pool(name="sbp", bufs=10) as sbp,        # exp(ST) tiles
            tc.tile_pool(name="sbr", bufs=4) as sbr,        # r chunks (f32r)
            tc.tile_pool(name="sbrv", bufs=2) as sbrv,      # rinv chunks
            tc.tile_pool(name="sbst", bufs=2) as sbst,      # B-head att stage
            tc.tile_pool(name="sbdp", bufs=2) as sbdp,      # dup q/k scratch
            tc.tile_pool(name="sbat", bufs=1) as sbat,      # att pairs
            tc.tile_pool(name="sbo", bufs=4) as sbo,        # out evac
            tc.tile_pool(name="psb", bufs=2, space="PSUM") as psb,   # [128,1024]
            tc.tile_pool(name="psa", bufs=3, space="PSUM") as psa,   # [128,512]
            tc.tile_pool(name="psc", bufs=1, space="PSUM") as psc,   # [128,512]
        ):
            # ---------- constants ----------
            ones_f = sbc.tile([128, 128], F32, name="ones_f")
            nc.gpsimd.memset(ones_f[:], 1.0)
            ones_r = sbc.tile([128, 128], F32R, name="ones_r")
            nc.vector.tensor_copy(ones_r[:], ones_f[:])

            # ---------- input DMA ----------
            xt_t = [sbin.tile([128, S], BF16, name=f"xt{k}", tag=f"xt{k}")
                    for k in range(KC)]
            wqk_t = [sbin.tile([128, D], BF16, name=f"wqk{k}", tag=f"wqk{k}")
                     for k in range(KC)]
            wv_t = [sbin.tile([128, NH * DH], BF16, name=f"wv{k}", tag=f"wv{k}")
                    for k in range(KC)]
            wout_t = [sbin.tile([128, D], BF16, name=f"wout{p}", tag=f"wout{p}")
                      for p in range(NPAIR)]
            # wv first (phase 1b needs it in full), xt next (streamed per
            # k-chunk), then wqk and wout
            for k in range(KC):
                nc.gpsimd.dma_start(wv_t[k][:], wv[k * 128:(k + 1) * 128, :])
            # xt lands s-chunk-major: phase 1b's first t-blocks need column
            # chunk 0 of every k-tile, so make them arrive first
            for sc in range(SC):
                for k in range(KC):
                    nc.sync.dma_start(
                        xt_t[k][:, sc * 512:(sc + 1) * 512],
                        xt[k * 128:(k + 1) * 128, sc * 512:(sc + 1) * 512])
            for k in range(KC):
                nc.sync.dma_start(wqk_t[k][:], wqk[k * 128:(k + 1) * 128, :])
            for p in range(NPAIR):
                nc.sync.dma_start(wout_t[p][:], wout[p * 128:(p + 1) * 128, :])

            # ---------- persistent intermediates ----------
            # qkT rows: tiles 0..3 = qT head pairs, 4..7 = kT head pairs
            qk_t = [sbqk.tile([128, S], BF16, name=f"qk{n}", tag=f"qk{n}")
                    for n in range(2 * NPAIR)]
            # v_aug: [t, 8*(64+1)]; per head 64 v columns then a ones column
            v_t = [sbv.tile([128, NH * (DH + 1)], BF16, name=f"v{t}", tag=f"v{t}")
                   for t in range(TT)]
            att_t = [sbat.tile([128, S], BF16, name=f"att{p}", tag=f"att{p}")
                     for p in range(NPAIR)]

            # ---------- phase 1b: v (natural layout) ----------
            for tb in range(TT):
                ps = psb.tile([128, 1024], F32, name=f"psv{tb}", tag="psb")
                for k in range(KC):
                    nc.tensor.matmul(
                        ps[:, 0:NH * DH],
                        xt_t[k][:, tb * 128:(tb + 1) * 128],
                        wv_t[k][:],
                        start=(k == 0),
                        stop=(k == KC - 1),
                    )
                nc.gpsimd.memset(v_t[tb][:], 1.0)
                nc.vector.tensor_copy(
                    v_t[tb][:].rearrange("p (h e) -> p h e", e=DH + 1)[:, :, 0:DH],
                    ps[:, 0:NH * DH].rearrange("p (h e) -> p h e", e=DH),
                )

            head_order = [1, 0, 3, 2, 5, 4, 7, 6]   # B before A per pair
            dup_q = {}
            dup_k = {}

            def emit_dup_half(h, sh):
                """Duplicate head h's q/k rows (s-columns sh*1024:+1024) to
                both partition halves so two t-tiles' ST matmuls row-pack
                (bases 0 and 64)."""
                pr, half = h // 2, h % 2
                r0, r1 = half * 64, half * 64 + 64
                if h not in dup_q:
                    dup_q[h] = sbdp.tile([128, S], BF16, name=f"qd{h}",
                                         tag="qd")
                    dup_k[h] = sbdp.tile([128, S], BF16, name=f"kd{h}",
                                         tag="kd")
                cols = slice(sh * 1024, (sh + 1) * 1024)
                for dst0 in (0, 64):
                    nc.sync.dma_start(dup_q[h][dst0:dst0 + 64, cols],
                                      qk_t[pr][r0:r1, cols])
                    nc.sync.dma_start(dup_k[h][dst0:dst0 + 64, cols],
                                      qk_t[NPAIR + pr][r0:r1, cols])

            def emit_dup(h):
                emit_dup_half(h, 0)
                emit_dup_half(h, 1)

            # ---------- phase 1a: qT / kT for head pair 0 (serial) ----------
            p1a_pair0 = ((NPAIR, 0), (0, 0), (NPAIR, 1), (0, 1))
            dup_after = {(0, 0): 0, (0, 1): 1}   # dup half after these units
            for nb, scp in p1a_pair0:
                ps = psb.tile([128, 1024], F32, name=f"psq{nb}_{scp}",
                              tag="psb")
                for half1 in range(2):
                    sc = 2 * scp + half1
                    for k in range(KC):
                        nc.tensor.matmul(
                            ps[:, half1 * 512:(half1 + 1) * 512],
                            wqk_t[k][:, nb * 128:(nb + 1) * 128],
                            xt_t[k][:, sc * 512:(sc + 1) * 512],
                            start=(k == 0),
                            stop=(k == KC - 1),
                        )
                nc.vector.tensor_copy(
                    qk_t[nb][:, scp * 1024:(scp + 1) * 1024], ps[:])
                if (nb, scp) in dup_after:
                    emit_dup_half(head_order[0], dup_after[(nb, scp)])

            # ---------- phase 2: attention (s-halved, software-pipelined) --
            # Flat stream over (head, s-half pass, t-tile). ScalarE exp is
            # the bottleneck; TensorE rides ~1 step of ST + lagged PV + one
            # drip-fed phase-1a matmul per step (pairs 1-3 of qkT computed
            # inside the attention stream through a dedicated PSUM bank).
            LAG = 3

            # injected phase-1a work: one unit = one (nb, s-chunk) 8-matmul
            # accumulation + cast, split into single-matmul thunks
            def make_inject_thunks():
                # (nb, sc) units, front-loaded: the dup-DMA prefetch for a
                # pair's heads reads the whole qkT tile, so pair p must be
                # complete ~8 tp-steps before its first head starts
                order = []
                releases = []
                u = 0
                for bi, base in enumerate((1, 2, 3)):
                    qn, kn = base, NPAIR + base
                    for sc in range(SC):
                        for nbx in (qn, kn):
                            order.append((nbx, sc))
                            # pair 1 front-loaded (its dup prefetch reads the
                            # whole tile at step 24); pairs 2/3 stretched to
                            # keep the PE fed through step ~85
                            if bi == 0:
                                releases.append(2 * u)
                            else:
                                releases.append(16 * bi + 4 * (u - 8 * bi))
                            u += 1
                thunks = []
                rel = []
                for u, (nb, sc) in enumerate(order):
                    state = {}

                    def mk_mm(k, nb=nb, sc=sc, state=state):
                        def run():
                            if "ps" not in state:
                                state["ps"] = psc.tile(
                                    [128, 512], F32,
                                    name=f"psi{nb}_{sc}", tag="psc")
                            nc.tensor.matmul(
                                state["ps"][:],
                                wqk_t[k][:, nb * 128:(nb + 1) * 128],
                                xt_t[k][:, sc * 512:(sc + 1) * 512],
                                start=(k == 0),
                                stop=(k == KC - 1),
                            )
                        return run

                    def mk_evac(nb=nb, sc=sc, state=state):
                        def run():
                            nc.vector.tensor_copy(
                                qk_t[nb][:, sc * 512:(sc + 1) * 512],
                                state["ps"][:])
                        return run

                    for k in range(KC):
                        thunks.append(mk_mm(k))
                        rel.append(releases[u] + k)
                    thunks.append(mk_evac())
                    rel.append(releases[u] + KC)
                return thunks, rel

            inject, inj_rel = make_inject_thunks()
            inj_pos = 0

            # HAM warm-keeper: harmless matmuls accumulating into a scratch
            # PSUM tile (flushed to a tiny scratch output so DCE keeps them).
            warm_state = {"n": 0, "ps": None}

            def emit_warm():
                if warm_state["ps"] is None:
                    warm_state["ps"] = psc.tile([128, 512], F32,
                                                name="pswarm", tag="psc")
                nc.tensor.matmul(
                    warm_state["ps"][:],
                    wqk_t[0][:, 0:128],
                    xt_t[0][:, 0:512],
                    start=(warm_state["n"] == 0),
                    stop=False,
                    skip_group_check=True,
                )
                warm_state["n"] += 1

            def flush_warm():
                if warm_state["ps"] is None:
                    return
                nc.tensor.matmul(
                    warm_state["ps"][:],
                    wqk_t[0][:, 0:128],
                    xt_t[0][:, 0:512],
                    start=False,
                    stop=True,
                    skip_group_check=True,
                )
                wsb = sbo.tile([1, 512], F32, name="warm_sb", tag="warm_sb")
                nc.vector.tensor_copy(wsb[:], warm_state["ps"][0:1, :])
                nc.sync.dma_start(warm[:, :], wsb[:])
                warm_state["ps"] = None

            p_tiles = {}    # (h, pa, tt) -> p tile
            accs = {}       # (h, pa) -> [2 psum accumulators]
            r_cs = {}       # (h, pa) -> [2 r tiles]
            stages = {}     # h -> B-head staging tile

            def retire(hp, pap, tpp):
                """Emit PV for t-tiles 2*tpp, 2*tpp+1; after the last pair
                emit the normalization tail inline."""
                h = hp
                if tpp == 0:
                    accs[(hp, pap)] = [
                        psa.tile([128, 512], F32, name=f"pv{hp}_{pap}_{cc}",
                                 tag="psa")
                        for cc in range(2)]
                acc = accs[(hp, pap)]
                for ttp in (2 * tpp, 2 * tpp + 1):
                    pt = p_tiles.pop((hp, pap, ttp))
                    for cc in range(2):
                        nc.tensor.matmul(
                            acc[cc][0:DH + 1, :],
                            v_t[ttp][:, h * (DH + 1):(h + 1) * (DH + 1)],
                            pt[:, cc * 512:(cc + 1) * 512],
                            start=(ttp == 0),
                            stop=(ttp == TT - 1),
                        )
                if tpp == TT // 2 - 1:
                    pr, half = hp // 2, hp % 2
                    if half == 0:
                        dst = att_t[pr]
                    else:
                        if pap == 0:
                            stages[hp] = sbst.tile([64, S], BF16,
                                                   name=f"bst{hp}",
                                                   tag="bstage")
                        dst = stages[hp]
                    for cc in range(2):
                        cg = pap * 2 + cc
                        r = sbr.tile([128, 512], F32R,
                                     name=f"r{hp}_{pap}_{cc}", tag="r")
                        nc.vector.tensor_copy(r[64:65, :], acc[cc][64:65, :])
                        rb = psa.tile([128, 512], F32,
                                      name=f"rb{hp}_{pap}_{cc}", tag="psa")
                        nc.tensor.matmul(
                            rb[0:64, :],
                            ones_r[64:65, 0:64],
                            r[64:65, :],
                            start=True,
                            stop=True,
                        )
                        rinv = sbrv.tile([64, 512], F32,
                                         name=f"rinv{hp}_{pap}_{cc}",
                                         tag="rinv")
                        nc.vector.reciprocal_approx_fast(rinv[:], rb[0:64, :])
                        nc.vector.tensor_mul(
                            dst[0:64, cg * 512:(cg + 1) * 512],
                            acc[cc][0:64, :],
                            rinv[:],
                        )
                    if half == 1 and pap == 1:
                        nc.sync.dma_start(att_t[pr][64:128, :],
                                          stages[hp][0:64, :])

            seq = [(h, pa, tp)
                   for h in head_order for pa in range(2)
                   for tp in range(TT // 2)]
            for i, (h, pa, tp) in enumerate(seq):
                qd, kd = dup_q[h], dup_k[h]
                tt0, tt1 = 2 * tp, 2 * tp + 1

                # emit PV/injection BEFORE the ST quad: the scheduler pops
                # ready work in priority (program) order, so nothing can
                # interleave into the row-packed A/B matmul pairs
                if pa == 1 and tp == 4:
                    # prefetch dup tiles for the next head (12 steps ahead;
                    # late enough that injected qkT writes precede the read)
                    hi = head_order.index(h)
                    if hi + 1 < len(head_order):
                        emit_dup(head_order[hi + 1])
                if i >= LAG:
                    retire(*seq[i - LAG])
                ran = 0
                while (inj_pos < len(inject) and inj_rel[inj_pos] <= i
                       and ran < 4):
                    inject[inj_pos]()
                    inj_pos += 1
                    ran += 1
                if ran == 0:
                    emit_warm()

                ptA = sbp.tile([128, 1024], BF16, name=f"p{h}_{pa}_{tt0}",
                               tag="p")
                ptB = sbp.tile([128, 1024], BF16, name=f"p{h}_{pa}_{tt1}",
                               tag="p")
                p_tiles[(h, pa, tt0)] = ptA
                p_tiles[(h, pa, tt1)] = ptB
                psA = psb.tile([128, 1024], F32, name=f"pstA{h}_{pa}_{tp}",
                               tag="psb")
                psB = psb.tile([128, 1024], F32, name=f"pstB{h}_{pa}_{tp}",
                               tag="psb")
                for j in range(2):
                    nc.tensor.matmul(
                        psA[:, j * 512:(j + 1) * 512],
                        kd[0:64, tt0 * 128:(tt0 + 1) * 128],
                        qd[0:64, pa * 1024 + j * 512:pa * 1024 + (j + 1) * 512],
                        start=True,
                        stop=True,
                    )
                    nc.tensor.matmul(
                        psB[:, j * 512:(j + 1) * 512],
                        kd[64:128, tt1 * 128:(tt1 + 1) * 128],
                        qd[64:128, pa * 1024 + j * 512:pa * 1024 + (j + 1) * 512],
                        start=True,
                        stop=True,
                    )
                nc.scalar.activation(
                    ptA[:], psA[:], mybir.ActivationFunctionType.Exp,
                    scale=0.125)
                nc.scalar.activation(
                    ptB[:], psB[:], mybir.ActivationFunctionType.Exp,
                    scale=0.125)
            while inj_pos < len(inject):
                inject[inj_pos]()
                inj_pos += 1
            for i in range(len(seq) - LAG, len(seq)):
                retire(*seq[i])
                for _ in range(6):
                    emit_warm()
            for _ in range(12):
                emit_warm()
            flush_warm()

            # ---------- phase 3: output projection ----------
            # attention pools are idle now; rotate psum groups across all
            # three pools for deeper pipelining
            for sb_i in range(TT):
                for e in range(2):
                    sel = (2 * sb_i + e) % 4
                    if sel < 2:
                        ps = psa.tile([128, 512], F32, name=f"po{sb_i}_{e}",
                                      tag="psa")
                    elif sel == 2:
                        psw = psb.tile([128, 1024], F32, name=f"po{sb_i}_{e}",
                                       tag="psb")
                        ps = psw[:, 0:512]
                    else:
                        ps = psc.tile([128, 512], F32, name=f"po{sb_i}_{e}",
                                      tag="psc")
                    for pr in range(NPAIR):
                        nc.tensor.matmul(
                            ps[:],
                            att_t[pr][:, sb_i * 128:(sb_i + 1) * 128],
                            wout_t[pr][:, e * 512:(e + 1) * 512],
                            start=(pr == 0),
                            stop=(pr == NPAIR - 1),
                        )
                    o = sbo.tile([128, 512], F32, name=f"o{sb_i}_{e}", tag="o")
                    nc.vector.tensor_copy(o[:], ps[:])
                    nc.sync.dma_start(
                        out_p[sb_i * 128:(sb_i + 1) * 128,
                              e * 512:(e + 1) * 512], o[:])

    nc.compile()
    return nc


def get_nc():
    global _CACHED_NC
    if _CACHED_NC is None:
        _CACHED_NC = build_nc()
    return _CACHED_NC


def make_in_maps(x, w_qkv, w_out):
    bf = ml_dtypes.bfloat16
    in_maps = []
    for c in range(N_CORES):
        b = c // 2
        h0 = (c % 2) * NH
        q_cols = w_qkv[:, h0 * DH:(h0 + NH) * DH]
        k_cols = w_qkv[:, D + h0 * DH:D + (h0 + NH) * DH]
        v_cols = w_qkv[:, 2 * D + h0 * DH:2 * D + (h0 + NH) * DH]
        in_maps.append({
            "xt": np.ascontiguousarray(x[b].T).astype(bf),
            "wqk": np.concatenate([q_cols, k_cols], axis=1).astype(bf),
            "wv": np.ascontiguousarray(v_cols).astype(bf),
            "wout": np.ascontiguousarray(
                w_out[h0 * DH:(h0 + NH) * DH, :]).astype(bf),
        })
    return in_maps


def run(x, w_qkv, w_out, trace=False, trace_cores=None):
    nc = get_nc()
    in_maps = make_in_maps(x, w_qkv, w_out)
    res = bass_utils.run_bass_kernel_spmd(
        nc, in_maps, core_ids=list(range(N_CORES)),
        trace=trace, trace_cores=trace_cores,
    )
    partials = [res.results[c]["out_p"] for c in range(N_CORES)]
    out = np.stack([partials[2 * b] + partials[2 * b + 1] for b in range(4)])
    return out.astype(np.float32), res


def kernel(x, w_qkv, w_out):
    out, _ = run(np.asarray(x), np.asarray(w_qkv), np.asarray(w_out))
    return out


if __name__ == "__main__":
    import tempfile
    nc = build_nc()
    with tempfile.TemporaryDirectory() as td:
        neff = bass_utils.compile_bass_kernel(nc, td)
        print("LOCAL COMPILE OK:", neff)


# revision 32
# speedup vs baseline: 1.0695x; 1.0002x over previous
"""Multi-head attention block (nn_AttentionBlock) on 8 Trainium2 NeuronCores.

Reference computation (fp32):
    qkv = x @ w_qkv;  q,k,v = split(qkv)
    per head: att = softmax(q @ k.T / 8) @ v
    out = concat_heads(att) @ w_out
Shapes: x [4, 2048, 1024], w_qkv [1024, 3072], w_out [1024, 1024], 16 heads.

Sharding: batch x head-half. Core c handles batch b = c//2 and heads
h0 = (c%2)*8 .. h0+8. Each core computes its 8 heads' attention and a partial
output projection (row-shard of w_out); host sums the two partials per batch.

Per-core kernel (all heavy matmuls bf16 with fp32 PSUM accumulation):
  P1: qkT[n, s] = w_qk.T @ x.T (weights stationary, xT moving) -> q/k heads
      transposed in SBUF; v[t, d] = x @ w_v in natural layout with an
      appended ones column per head (v_aug).
  P2 per head pair (2 heads share a 128-partition tile, K=64 row-packed
      matmuls at base partitions 0/64):
      ST[t, s] = exp((k_t . q_s)/8)  via TensorE + ScalarE exp (psum->sbuf
      bf16), no max subtraction (logits are O(5), exp is safe in fp32).
      PV: att_aug[d|1, s] = v_aug.T @ p accumulated over 16 t-chunks; row 64
      is r[s] = sum_t p. rinv broadcast via K=1 ones matmul + reciprocal;
      normalize att rows with a tensor-tensor multiply.
  P3: partial[s, e] = att.T @ w_out accumulated over the 4 head pairs.
"""

import sys

sys.path.insert(0, "/opt/trn_rl_repo")

import numpy as np
import ml_dtypes

import concourse.mybir as mybir
import concourse.tile as tile
from concourse import bacc
from concourse import bass_utils

F32 = mybir.dt.float32
F32R = mybir.dt.float32r
BF16 = mybir.dt.bfloat16

S = 2048          # sequence length
D = 1024          # embed dim
DH = 64           # head dim
NH = 8            # heads per core
NPAIR = 4         # head pairs per core
KC = D // 128     # contraction chunks for phase 1
SC = S // 512     # 512-wide free chunks of the sequence
TT = S // 128     # 128-row t tiles of the sequence
N_CORES = 8

_CACHED_NC = None


def build_nc():
    nc = bacc.Bacc("TRN2", target_bir_lowering=False, debug=False,
                   num_devices=N_CORES)

    xt = nc.dram_tensor("xt", [D, S], BF16, kind="ExternalInput").ap()
    wqk = nc.dram_tensor("wqk", [D, D], BF16, kind="ExternalInput").ap()
    wv = nc.dram_tensor("wv", [D, NH * DH], BF16, kind="ExternalInput").ap()
    wout = nc.dram_tensor("wout", [NH * DH, D], BF16, kind="ExternalInput").ap()
    out_p = nc.dram_tensor("out_p", [S, D], F32, kind="ExternalOutput").ap()
    warm = nc.dram_tensor("warm", [1, 512], F32, kind="ExternalOutput").ap()

    with tile.TileContext(nc) as tc:
        with (
            tc.tile_pool(name="sbc", bufs=1) as sbc,        # constants
            tc.tile_pool(name="sbin", bufs=1) as sbin,      # phase-1 inputs
            tc.tile_pool(name="sbqk", bufs=1) as sbqk,      # qkT persistent
            tc.tile_pool(name="sbv", bufs=1) as sbv,        # v_aug persistent
            tc.tile_pool(name="sbp", bufs=10) as sbp,        # exp(ST) tiles
            tc.tile_pool(name="sbr", bufs=4) as sbr,        # r chunks (f32r)
            tc.tile_pool(name="sbrv", bufs=2) as sbrv,      # rinv chunks
            tc.tile_pool(name="sbst", bufs=2) as sbst,      # B-head att stage
            tc.tile_pool(name="sbdp", bufs=2) as sbdp,      # dup q/k scratch
            tc.tile_pool(name="sbat", bufs=1) as sbat,      # att pairs
            tc.tile_pool(name="sbo", bufs=4) as sbo,        # out evac
            tc.tile_pool(name="psb", bufs=2, space="PSUM") as psb,   # [128,1024]
            tc.tile_pool(name="psa", bufs=3, space="PSUM") as psa,   # [128,512]
            tc.tile_pool(name="psc", bufs=1, space="PSUM") as psc,   # [128,512]
        ):
            # ---------- constants ----------
            ones_f = sbc.tile([128, 128], F32, name="ones_f")
            nc.gpsimd.memset(ones_f[:], 1.0)
            ones_r = sbc.tile([128, 128], F32R, name="ones_r")
            nc.vector.tensor_copy(ones_r[:], ones_f[:])
            # warm the gpsimd ext-isa library (IRAM load ~6us) before phase 2
            # needs partition_broadcast on the normalization path
            pbw = sbc.tile([64, 32], F32, name="pb_warm")
            nc.gpsimd.memset(pbw[:], 0.0)
            nc.gpsimd.partition_broadcast(pbw[:, :], pbw[0:1, :])

            # ---------- input DMA ----------
            xt_t = [sbin.tile([128, S], BF16, name=f"xt{k}", tag=f"xt{k}")
                    for k in range(KC)]
            wqk_t = [sbin.tile([128, D], BF16, name=f"wqk{k}", tag=f"wqk{k}")
                     for k in range(KC)]
            wv_t = [sbin.tile([128, NH * DH], BF16, name=f"wv{k}", tag=f"wv{k}")
                    for k in range(KC)]
            wout_t = [sbin.tile([128, D], BF16, name=f"wout{p}", tag=f"wout{p}")
                      for p in range(NPAIR)]
            # wv first (phase 1b needs it in full), xt next (streamed per
            # k-chunk), then wqk and wout
            for k in range(KC):
                nc.gpsimd.dma_start(wv_t[k][:], wv[k * 128:(k + 1) * 128, :])
            # xt lands s-chunk-major: phase 1b's first t-blocks need column
            # chunk 0 of every k-tile, so make them arrive first
            for sc in range(SC):
                for k in range(KC):
                    nc.sync.dma_start(
                        xt_t[k][:, sc * 512:(sc + 1) * 512],
                        xt[k * 128:(k + 1) * 128, sc * 512:(sc + 1) * 512])
            for k in range(KC):
                nc.sync.dma_start(wqk_t[k][:], wqk[k * 128:(k + 1) * 128, :])
            for p in range(NPAIR):
                nc.sync.dma_start(wout_t[p][:], wout[p * 128:(p + 1) * 128, :])

            # ---------- persistent intermediates ----------
            # qkT rows: tiles 0..3 = qT head pairs, 4..7 = kT head pairs
            qk_t = [sbqk.tile([128, S], BF16, name=f"qk{n}", tag=f"qk{n}")
                    for n in range(2 * NPAIR)]
            # v_aug: [t, 8*(64+1)]; per head 64 v columns then a ones column
            v_t = [sbv.tile([128, NH * (DH + 1)], BF16, name=f"v{t}", tag=f"v{t}")
                   for t in range(TT)]
            att_t = [sbat.tile([128, S], BF16, name=f"att{p}", tag=f"att{p}")
                     for p in range(NPAIR)]

            # ---------- phase 1b: v (natural layout) ----------
            for tb in range(TT):
                ps = psb.tile([128, 1024], F32, name=f"psv{tb}", tag="psb")
                for k in range(KC):
                    nc.tensor.matmul(
                        ps[:, 0:NH * DH],
                        xt_t[k][:, tb * 128:(tb + 1) * 128],
                        wv_t[k][:],
                        start=(k == 0),
                        stop=(k == KC - 1),
                    )
                nc.gpsimd.memset(v_t[tb][:], 1.0)
                nc.vector.tensor_copy(
                    v_t[tb][:].rearrange("p (h e) -> p h e", e=DH + 1)[:, :, 0:DH],
                    ps[:, 0:NH * DH].rearrange("p (h e) -> p h e", e=DH),
                )

            head_order = [1, 0, 3, 2, 5, 4, 7, 6]   # B before A per pair
            dup_q = {}
            dup_k = {}

            def emit_dup_half(h, sh):
                """Duplicate head h's q/k rows (s-columns sh*1024:+1024) to
                both partition halves so two t-tiles' ST matmuls row-pack
                (bases 0 and 64)."""
                pr, half = h // 2, h % 2
                r0, r1 = half * 64, half * 64 + 64
                if h not in dup_q:
                    dup_q[h] = sbdp.tile([128, S], BF16, name=f"qd{h}",
                                         tag="qd")
                    dup_k[h] = sbdp.tile([128, S], BF16, name=f"kd{h}",
                                         tag="kd")
                cols = slice(sh * 1024, (sh + 1) * 1024)
                for dst0 in (0, 64):
                    nc.sync.dma_start(dup_q[h][dst0:dst0 + 64, cols],
                                      qk_t[pr][r0:r1, cols])
                    nc.sync.dma_start(dup_k[h][dst0:dst0 + 64, cols],
                                      qk_t[NPAIR + pr][r0:r1, cols])

            def emit_dup(h):
                emit_dup_half(h, 0)
                emit_dup_half(h, 1)

            # ---------- phase 1a: qT / kT for head pair 0 (serial) ----------
            p1a_pair0 = ((NPAIR, 0), (0, 0), (NPAIR, 1), (0, 1))
            dup_after = {(0, 0): 0, (0, 1): 1}   # dup half after these units
            for nb, scp in p1a_pair0:
                ps = psb.tile([128, 1024], F32, name=f"psq{nb}_{scp}",
                              tag="psb")
                for half1 in range(2):
                    sc = 2 * scp + half1
                    for k in range(KC):
                        nc.tensor.matmul(
                            ps[:, half1 * 512:(half1 + 1) * 512],
                            wqk_t[k][:, nb * 128:(nb + 1) * 128],
                            xt_t[k][:, sc * 512:(sc + 1) * 512],
                            start=(k == 0),
                            stop=(k == KC - 1),
                        )
                nc.vector.tensor_copy(
                    qk_t[nb][:, scp * 1024:(scp + 1) * 1024], ps[:])
                if (nb, scp) in dup_after:
                    emit_dup_half(head_order[0], dup_after[(nb, scp)])

            # ---------- phase 2: attention (s-halved, software-pipelined) --
            # Flat stream over (head, s-half pass, t-tile). ScalarE exp is
            # the bottleneck; TensorE rides ~1 step of ST + lagged PV + one
            # drip-fed phase-1a matmul per step (pairs 1-3 of qkT computed
            # inside the attention stream through a dedicated PSUM bank).
            LAG = 3

            # injected phase-1a work: one unit = one (nb, s-chunk) 8-matmul
            # accumulation + cast, split into single-matmul thunks
            def make_inject_thunks():
                # (nb, sc) units, front-loaded: the dup-DMA prefetch for a
                # pair's heads reads the whole qkT tile, so pair p must be
                # complete ~8 tp-steps before its first head starts
                order = []
                releases = []
                u = 0
                for bi, base in enumerate((1, 2, 3)):
                    qn, kn = base, NPAIR + base
                    for sc in range(SC):
                        for nbx in (qn, kn):
                            order.append((nbx, sc))
                            # pair 1 front-loaded (its dup prefetch reads the
                            # whole tile at step 24); pairs 2/3 stretched to
                            # keep the PE fed through step ~85
                            if bi == 0:
                                releases.append(2 * u)
                            else:
                                releases.append(16 * bi + 4 * (u - 8 * bi))
                            u += 1
                thunks = []
                rel = []
                for u, (nb, sc) in enumerate(order):
                    state = {}

                    def mk_mm(k, nb=nb, sc=sc, state=state):
                        def run():
                            if "ps" not in state:
                                state["ps"] = psc.tile(
                                    [128, 512], F32,
                                    name=f"psi{nb}_{sc}", tag="psc")
                            nc.tensor.matmul(
                                state["ps"][:],
                                wqk_t[k][:, nb * 128:(nb + 1) * 128],
                                xt_t[k][:, sc * 512:(sc + 1) * 512],
                                start=(k == 0),
                                stop=(k == KC - 1),
                            )
                        return run

                    def mk_evac(nb=nb, sc=sc, state=state):
                        def run():
                            nc.vector.tensor_copy(
                                qk_t[nb][:, sc * 512:(sc + 1) * 512],
                                state["ps"][:])
                        return run

                    for k in range(KC):
                        thunks.append(mk_mm(k))
                        rel.append(releases[u] + k)
                    thunks.append(mk_evac())
                    rel.append(releases[u] + KC)
                return thunks, rel

            inject, inj_rel = make_inject_thunks()
            inj_pos = 0

            # HAM warm-keeper: harmless matmuls accumulating into a scratch
            # PSUM tile (flushed to a tiny scratch output so DCE keeps them).
            warm_state = {"n": 0, "ps": None}

            def emit_warm():
                if warm_state["ps"] is None:
                    warm_state["ps"] = psc.tile([128, 512], F32,
                                                name="pswarm", tag="psc")
                nc.tensor.matmul(
                    warm_state["ps"][:],
                    wqk_t[0][:, 0:128],
                    xt_t[0][:, 0:512],
                    start=(warm_state["n"] == 0),
                    stop=False,
                    skip_group_check=True,
                )
                warm_state["n"] += 1

            def flush_warm():
                if warm_state["ps"] is None:
                    return
                nc.tensor.matmul(
                    warm_state["ps"][:],
                    wqk_t[0][:, 0:128],
                    xt_t[0][:, 0:512],
                    start=False,
                    stop=True,
                    skip_group_check=True,
                )
                wsb = sbo.tile([1, 512], F32, name="warm_sb", tag="warm_sb")
                nc.vector.tensor_copy(wsb[:], warm_state["ps"][0:1, :])
                nc.sync.dma_start(warm[:, :], wsb[:])
                warm_state["ps"] = None

            p_tiles = {}    # (h, pa, tt) -> p tile
            accs = {}       # (h, pa) -> [2 psum accumulators]
            r_cs = {}       # (h, pa) -> [2 r tiles]
            stages = {}     # h -> B-head staging tile

            def retire(hp, pap, tpp):
                """Emit PV for t-tiles 2*tpp, 2*tpp+1; after the last pair
                emit the normalization tail inline."""
                h = hp
                if tpp == 0:
                    accs[(hp, pap)] = [
                        psa.tile([128, 512], F32, name=f"pv{hp}_{pap}_{cc}",
                                 tag="psa")
                        for cc in range(2)]
                acc = accs[(hp, pap)]
                for ttp in (2 * tpp, 2 * tpp + 1):
                    pt = p_tiles.pop((hp, pap, ttp))
                    for cc in range(2):
                        nc.tensor.matmul(
                            acc[cc][0:DH + 1, :],
                            v_t[ttp][:, h * (DH + 1):(h + 1) * (DH + 1)],
                            pt[:, cc * 512:(cc + 1) * 512],
                            start=(ttp == 0),
                            stop=(ttp == TT - 1),
                        )
                if tpp == TT // 2 - 1:
                    pr, half = hp // 2, hp % 2
                    if half == 0:
                        dst = att_t[pr]
                    else:
                        if pap == 0:
                            stages[hp] = sbst.tile([64, S], BF16,
                                                   name=f"bst{hp}",
                                                   tag="bstage")
                        dst = stages[hp]
                    for cc in range(2):
                        cg = pap * 2 + cc
                        # r row -> partition 0 of an sbuf tile, gpsimd
                        # broadcasts it to all 64 partitions (replaces a K=1
                        # PE matmul + its weight-register thrash)
                        rbc = sbr.tile([64, 512], F32,
                                       name=f"r{hp}_{pap}_{cc}", tag="r")
                        nc.vector.tensor_copy(rbc[0:1, :], acc[cc][64:65, :])
                        nc.gpsimd.partition_broadcast(rbc[:, :], rbc[0:1, :])
                        rinv = sbrv.tile([64, 512], F32,
                                         name=f"rinv{hp}_{pap}_{cc}",
                                         tag="rinv")
                        nc.vector.reciprocal_approx_fast(rinv[:], rbc[:])
                        nc.vector.tensor_mul(
                            dst[0:64, cg * 512:(cg + 1) * 512],
                            acc[cc][0:64, :],
                            rinv[:],
                        )
                    if half == 1 and pap == 1:
                        nc.sync.dma_start(att_t[pr][64:128, :],
                                          stages[hp][0:64, :])

            seq = [(h, pa, tp)
                   for h in head_order for pa in range(2)
                   for tp in range(TT // 2)]
            for i, (h, pa, tp) in enumerate(seq):
                qd, kd = dup_q[h], dup_k[h]
                tt0, tt1 = 2 * tp, 2 * tp + 1

                # emit PV/injection BEFORE the ST quad: the scheduler pops
                # ready work in priority (program) order, so nothing can
                # interleave into the row-packed A/B matmul pairs
                if pa == 1 and tp == 4:
                    # prefetch dup tiles for the next head (12 steps ahead;
                    # late enough that injected qkT writes precede the read)
                    hi = head_order.index(h)
                    if hi + 1 < len(head_order):
                        emit_dup(head_order[hi + 1])
                if i >= LAG:
                    retire(*seq[i - LAG])
                ran = 0
                while (inj_pos < len(inject) and inj_rel[inj_pos] <= i
                       and ran < 4):
                    inject[inj_pos]()
                    inj_pos += 1
                    ran += 1
                if ran == 0:
                    emit_warm()

                ptA = sbp.tile([128, 1024], BF16, name=f"p{h}_{pa}_{tt0}",
                               tag="p")
                ptB = sbp.tile([128, 1024], BF16, name=f"p{h}_{pa}_{tt1}",
                               tag="p")
                p_tiles[(h, pa, tt0)] = ptA
                p_tiles[(h, pa, tt1)] = ptB
                psA = psb.tile([128, 1024], F32, name=f"pstA{h}_{pa}_{tp}",
                               tag="psb")
                psB = psb.tile([128, 1024], F32, name=f"pstB{h}_{pa}_{tp}",
                               tag="psb")
                for j in range(2):
                    nc.tensor.matmul(
                        psA[:, j * 512:(j + 1) * 512],
                        kd[0:64, tt0 * 128:(tt0 + 1) * 128],
                        qd[0:64, pa * 1024 + j * 512:pa * 1024 + (j + 1) * 512],
                        start=True,
                        stop=True,
                    )
                    nc.tensor.matmul(
                        psB[:, j * 512:(j + 1) * 512],
                        kd[64:128, tt1 * 128:(tt1 + 1) * 128],
                        qd[64:128, pa * 1024 + j * 512:pa * 1024 + (j + 1) * 512],
                        start=True,
                        stop=True,
                    )
                nc.scalar.activation(
                    ptA[:], psA[:], mybir.ActivationFunctionType.Exp,
                    scale=0.125)
                nc.scalar.activation(
                    ptB[:], psB[:], mybir.ActivationFunctionType.Exp,
                    scale=0.125)
            while inj_pos < len(inject):
                inject[inj_pos]()
                inj_pos += 1
            for i in range(len(seq) - LAG, len(seq)):
                retire(*seq[i])
                for _ in range(6):
                    emit_warm()
            for _ in range(12):
                emit_warm()
            flush_warm()

            # ---------- phase 3: output projection ----------
            # attention pools are idle now; rotate psum groups across all
            # three pools for deeper pipelining
            for sb_i in range(TT):
                for e in range(2):
                    sel = (2 * sb_i + e) % 4
                    if sel < 2:
                        ps = psa.tile([128, 512], F32, name=f"po{sb_i}_{e}",
                                      tag="psa")
                    elif sel == 2:
                        psw = psb.tile([128, 1024], F32, name=f"po{sb_i}_{e}",
                                       tag="psb")
                        ps = psw[:, 0:512]
                    else:
                        ps = psc.tile([128, 512], F32, name=f"po{sb_i}_{e}",
                                      tag="psc")
                    for pr in range(NPAIR):
                        nc.tensor.matmul(
                            ps[:],
                            att_t[pr][:, sb_i * 128:(sb_i + 1) * 128],
                            wout_t[pr][:, e * 512:(e + 1) * 512],
                            start=(pr == 0),
                            stop=(pr == NPAIR - 1),
                        )
                    o = sbo.tile([128, 512], F32, name=f"o{sb_i}_{e}", tag="o")
                    nc.vector.tensor_copy(o[:], ps[:])
                    nc.sync.dma_start(
                        out_p[sb_i * 128:(sb_i + 1) * 128,
                              e * 512:(e + 1) * 512], o[:])

    nc.compile()
    return nc


def get_nc():
    global _CACHED_NC
    if _CACHED_NC is None:
        _CACHED_NC = build_nc()
    return _CACHED_NC


def make_in_maps(x, w_qkv, w_out):
    bf = ml_dtypes.bfloat16
    in_maps = []
    for c in range(N_CORES):
        b = c // 2
        h0 = (c % 2) * NH
        q_cols = w_qkv[:, h0 * DH:(h0 + NH) * DH]
        k_cols = w_qkv[:, D + h0 * DH:D + (h0 + NH) * DH]
        v_cols = w_qkv[:, 2 * D + h0 * DH:2 * D + (h0 + NH) * DH]
        in_maps.append({
            "xt": np.ascontiguousarray(x[b].T).astype(bf),
            "wqk": np.concatenate([q_cols, k_cols], axis=1).astype(bf),
            "wv": np.ascontiguousarray(v_cols).astype(bf),
            "wout": np.ascontiguousarray(
                w_out[h0 * DH:(h0 + NH) * DH, :]).astype(bf),
        })
    return in_maps


def run(x, w_qkv, w_out, trace=False, trace_cores=None):
    nc = get_nc()
    in_maps = make_in_maps(x, w_qkv, w_out)
    res = bass_utils.run_bass_kernel_spmd(
        nc, in_maps, core_ids=list(range(N_CORES)),
        trace=trace, trace_cores=trace_cores,
    )
    partials = [res.results[c]["out_p"] for c in range(N_CORES)]
    out = np.stack([partials[2 * b] + partials[2 * b + 1] for b in range(4)])
    return out.astype(np.float32), res


def kernel(x, w_qkv, w_out):
    out, _ = run(np.asarray(x), np.asarray(w_qkv), np.asarray(w_out))
    return out


if __name__ == "__main__":
    import tempfile
    nc = build_nc()
    with tempfile.TemporaryDirectory() as td:
        neff = bass_utils.compile_bass_kernel(nc, td)
        print("LOCAL COMPILE OK:", neff)


# revision 42
# speedup vs baseline: 1.0925x; 1.0215x over previous
"""Multi-head attention block (nn_AttentionBlock) on 8 Trainium2 NeuronCores.

Reference computation (fp32):
    qkv = x @ w_qkv;  q,k,v = split(qkv)
    per head: att = softmax(q @ k.T / 8) @ v
    out = concat_heads(att) @ w_out
Shapes: x [4, 2048, 1024], w_qkv [1024, 3072], w_out [1024, 1024], 16 heads.

Sharding: batch x head-half. Core c handles batch b = c//2 and heads
h0 = (c%2)*8 .. h0+8. Each core computes its 8 heads' attention and a partial
output projection (row-shard of w_out); host sums the two partials per batch.

Per-core kernel (all heavy matmuls bf16 with fp32 PSUM accumulation):
  P1: qkT[n, s] = w_qk.T @ x.T (weights stationary, xT moving) -> q/k heads
      transposed in SBUF; v[t, d] = x @ w_v in natural layout with an
      appended ones column per head (v_aug).
  P2 per head pair (2 heads share a 128-partition tile, K=64 row-packed
      matmuls at base partitions 0/64):
      ST[t, s] = exp((k_t . q_s)/8)  via TensorE + ScalarE exp (psum->sbuf
      bf16), no max subtraction (logits are O(5), exp is safe in fp32).
      PV: att_aug[d|1, s] = v_aug.T @ p accumulated over 16 t-chunks; row 64
      is r[s] = sum_t p. rinv broadcast via K=1 ones matmul + reciprocal;
      normalize att rows with a tensor-tensor multiply.
  P3: partial[s, e] = att.T @ w_out accumulated over the 4 head pairs.
"""

import sys

sys.path.insert(0, "/opt/trn_rl_repo")

import numpy as np
import ml_dtypes

import concourse.mybir as mybir
import concourse.tile as tile
from concourse import bacc
from concourse import bass_utils

F32 = mybir.dt.float32
F32R = mybir.dt.float32r
BF16 = mybir.dt.bfloat16

S = 2048          # sequence length
D = 1024          # embed dim
DH = 64           # head dim
NH = 8            # heads per core
NPAIR = 4         # head pairs per core
KC = D // 128     # contraction chunks for phase 1
SC = S // 512     # 512-wide free chunks of the sequence
TT = S // 128     # 128-row t tiles of the sequence
N_CORES = 8

_CACHED_NC = None


def build_nc():
    nc = bacc.Bacc("TRN2", target_bir_lowering=False, debug=False,
                   num_devices=N_CORES)

    xt = nc.dram_tensor("xt", [D, S], BF16, kind="ExternalInput").ap()
    wqk = nc.dram_tensor("wqk", [D, D], BF16, kind="ExternalInput").ap()
    wv = nc.dram_tensor("wv", [D, NH * DH], BF16, kind="ExternalInput").ap()
    wout = nc.dram_tensor("wout", [NH * DH, D], BF16, kind="ExternalInput").ap()
    # bf16 partials: halves the 8MB output DMA on the tail; the host sums
    # the two per-batch partials in f32 (adds ~0.3% rel err, budget is 2%)
    out_p = nc.dram_tensor("out_p", [S, D], BF16, kind="ExternalOutput").ap()
    warm = nc.dram_tensor("warm", [1, 512], F32, kind="ExternalOutput").ap()

    with tile.TileContext(nc) as tc:
        with (
            tc.tile_pool(name="sbc", bufs=1) as sbc,        # constants
            tc.tile_pool(name="sbin", bufs=1) as sbin,      # phase-1 inputs
            tc.tile_pool(name="sbqk", bufs=1) as sbqk,      # qkT persistent
            tc.tile_pool(name="sbv", bufs=1) as sbv,        # v_aug persistent
            tc.tile_pool(name="sbp", bufs=10) as sbp,        # exp(ST) tiles
            tc.tile_pool(name="sbr", bufs=4) as sbr,        # r chunks (f32r)
            tc.tile_pool(name="sbrv", bufs=2) as sbrv,      # rinv chunks
            tc.tile_pool(name="sbst", bufs=2) as sbst,      # B-head att stage
            tc.tile_pool(name="sbdp", bufs=2) as sbdp,      # dup q/k scratch
            tc.tile_pool(name="sbat", bufs=1) as sbat,      # att pairs
            tc.tile_pool(name="sbo", bufs=4) as sbo,        # out evac
            tc.tile_pool(name="psb", bufs=2, space="PSUM") as psb,   # [128,1024]
            tc.tile_pool(name="psa", bufs=3, space="PSUM") as psa,   # [128,512]
            tc.tile_pool(name="psc", bufs=1, space="PSUM") as psc,   # [128,512]
        ):
            # ---------- constants ----------
            ones_f = sbc.tile([128, 128], F32, name="ones_f")
            nc.gpsimd.memset(ones_f[:], 1.0)
            ones_r = sbc.tile([128, 128], F32R, name="ones_r")
            nc.vector.tensor_copy(ones_r[:], ones_f[:])

            # ---------- input DMA ----------
            xt_t = [sbin.tile([128, S], BF16, name=f"xt{k}", tag=f"xt{k}")
                    for k in range(KC)]
            wqk_t = [sbin.tile([128, D], BF16, name=f"wqk{k}", tag=f"wqk{k}")
                     for k in range(KC)]
            wv_t = [sbin.tile([128, NH * DH], BF16, name=f"wv{k}", tag=f"wv{k}")
                    for k in range(KC)]
            wout_t = [sbin.tile([128, D], BF16, name=f"wout{p}", tag=f"wout{p}")
                      for p in range(NPAIR)]
            # wv first (phase 1b needs it in full), xt next (streamed per
            # k-chunk), then wqk and wout
            for k in range(KC):
                nc.gpsimd.dma_start(wv_t[k][:], wv[k * 128:(k + 1) * 128, :])
            # xt lands s-chunk-major: phase 1b's first t-blocks need column
            # chunk 0 of every k-tile, so make them arrive first
            for sc in range(SC):
                for k in range(KC):
                    nc.sync.dma_start(
                        xt_t[k][:, sc * 512:(sc + 1) * 512],
                        xt[k * 128:(k + 1) * 128, sc * 512:(sc + 1) * 512])
            for k in range(KC):
                nc.sync.dma_start(wqk_t[k][:], wqk[k * 128:(k + 1) * 128, :])
            for p in range(NPAIR):
                nc.sync.dma_start(wout_t[p][:], wout[p * 128:(p + 1) * 128, :])
            # warm the gpsimd ext-isa library (IRAM load ~6us) well before
            # phase 2 needs partition_broadcast; AFTER the dma emission so
            # the load doesn't delay the wv transfers on the gpsimd queue
            pbw = sbc.tile([64, 32], F32, name="pb_warm")
            nc.gpsimd.memset(pbw[:], 0.0)
            nc.gpsimd.partition_broadcast(pbw[:, :], pbw[0:1, :])

            # ---------- persistent intermediates ----------
            # qkT rows: tiles 0..3 = qT head pairs, 4..7 = kT head pairs
            qk_t = [sbqk.tile([128, S], BF16, name=f"qk{n}", tag=f"qk{n}")
                    for n in range(2 * NPAIR)]
            # v_aug: [t, 8*(64+1)]; per head 64 v columns then a ones column
            v_t = [sbv.tile([128, NH * (DH + 1)], BF16, name=f"v{t}", tag=f"v{t}")
                   for t in range(TT)]
            att_t = [sbat.tile([128, S], BF16, name=f"att{p}", tag=f"att{p}")
                     for p in range(NPAIR)]

            # ---------- phase 1b: v (natural layout) ----------
            for tb in range(TT):
                ps = psb.tile([128, 1024], F32, name=f"psv{tb}", tag="psb")
                for k in range(KC):
                    nc.tensor.matmul(
                        ps[:, 0:NH * DH],
                        xt_t[k][:, tb * 128:(tb + 1) * 128],
                        wv_t[k][:],
                        start=(k == 0),
                        stop=(k == KC - 1),
                    )
                nc.gpsimd.memset(v_t[tb][:], 1.0)
                nc.vector.tensor_copy(
                    v_t[tb][:].rearrange("p (h e) -> p h e", e=DH + 1)[:, :, 0:DH],
                    ps[:, 0:NH * DH].rearrange("p (h e) -> p h e", e=DH),
                )

            head_order = [1, 0, 3, 2, 5, 4, 7, 6]   # B before A per pair
            dup_q = {}
            dup_k = {}

            def emit_dup_half(h, sh):
                """Duplicate head h's q/k rows (s-columns sh*1024:+1024) to
                both partition halves so two t-tiles' ST matmuls row-pack
                (bases 0 and 64)."""
                pr, half = h // 2, h % 2
                r0, r1 = half * 64, half * 64 + 64
                if h not in dup_q:
                    dup_q[h] = sbdp.tile([128, S], BF16, name=f"qd{h}",
                                         tag="qd")
                    dup_k[h] = sbdp.tile([128, S], BF16, name=f"kd{h}",
                                         tag="kd")
                cols = slice(sh * 1024, (sh + 1) * 1024)
                for dst0 in (0, 64):
                    nc.sync.dma_start(dup_q[h][dst0:dst0 + 64, cols],
                                      qk_t[pr][r0:r1, cols])
                    nc.sync.dma_start(dup_k[h][dst0:dst0 + 64, cols],
                                      qk_t[NPAIR + pr][r0:r1, cols])

            def emit_dup(h):
                emit_dup_half(h, 0)
                emit_dup_half(h, 1)

            # ---------- phase 1a: qT / kT for head pair 0 (serial) ----------
            p1a_pair0 = ((NPAIR, 0), (0, 0), (NPAIR, 1), (0, 1))
            dup_after = {(0, 0): 0, (0, 1): 1}   # dup half after these units
            for nb, scp in p1a_pair0:
                ps = psb.tile([128, 1024], F32, name=f"psq{nb}_{scp}",
                              tag="psb")
                for half1 in range(2):
                    sc = 2 * scp + half1
                    for k in range(KC):
                        nc.tensor.matmul(
                            ps[:, half1 * 512:(half1 + 1) * 512],
                            wqk_t[k][:, nb * 128:(nb + 1) * 128],
                            xt_t[k][:, sc * 512:(sc + 1) * 512],
                            start=(k == 0),
                            stop=(k == KC - 1),
                        )
                nc.vector.tensor_copy(
                    qk_t[nb][:, scp * 1024:(scp + 1) * 1024], ps[:])
                if (nb, scp) in dup_after:
                    emit_dup_half(head_order[0], dup_after[(nb, scp)])

            # ---------- phase 2: attention (s-halved, software-pipelined) --
            # Flat stream over (head, s-half pass, t-tile). ScalarE exp is
            # the bottleneck; TensorE rides ~1 step of ST + lagged PV + one
            # drip-fed phase-1a matmul per step (pairs 1-3 of qkT computed
            # inside the attention stream through a dedicated PSUM bank).
            LAG = 3

            # injected phase-1a work: one unit = one (nb, s-chunk) 8-matmul
            # accumulation + cast, split into single-matmul thunks
            def make_inject_thunks():
                # (nb, sc) units, front-loaded: the dup-DMA prefetch for a
                # pair's heads reads the whole qkT tile, so pair p must be
                # complete ~8 tp-steps before its first head starts
                order = []
                releases = []
                u = 0
                for bi, base in enumerate((1, 2, 3)):
                    qn, kn = base, NPAIR + base
                    for sc in range(SC):
                        for nbx in (qn, kn):
                            order.append((nbx, sc))
                            # pair 1 front-loaded (its dup prefetch reads the
                            # whole tile at step 24); pairs 2/3 stretched to
                            # keep the PE fed through step ~85
                            if bi == 0:
                                releases.append(2 * u)
                            else:
                                releases.append(16 * bi + 4 * (u - 8 * bi))
                            u += 1
                thunks = []
                rel = []
                for u, (nb, sc) in enumerate(order):
                    state = {}

                    def mk_mm(k, nb=nb, sc=sc, state=state):
                        def run():
                            if "ps" not in state:
                                state["ps"] = psc.tile(
                                    [128, 512], F32,
                                    name=f"psi{nb}_{sc}", tag="psc")
                            nc.tensor.matmul(
                                state["ps"][:],
                                wqk_t[k][:, nb * 128:(nb + 1) * 128],
                                xt_t[k][:, sc * 512:(sc + 1) * 512],
                                start=(k == 0),
                                stop=(k == KC - 1),
                            )
                        return run

                    def mk_evac(nb=nb, sc=sc, state=state):
                        def run():
                            nc.vector.tensor_copy(
                                qk_t[nb][:, sc * 512:(sc + 1) * 512],
                                state["ps"][:])
                        return run

                    for k in range(KC):
                        thunks.append(mk_mm(k))
                        rel.append(releases[u] + k)
                    thunks.append(mk_evac())
                    rel.append(releases[u] + KC)
                return thunks, rel

            inject, inj_rel = make_inject_thunks()
            inj_pos = 0

            # HAM warm-keeper: harmless matmuls accumulating into a scratch
            # PSUM tile (flushed to a tiny scratch output so DCE keeps them).
            warm_state = {"n": 0, "ps": None}

            def emit_warm():
                if warm_state["ps"] is None:
                    warm_state["ps"] = psc.tile([128, 512], F32,
                                                name="pswarm", tag="psc")
                nc.tensor.matmul(
                    warm_state["ps"][:],
                    wqk_t[0][:, 0:128],
                    xt_t[0][:, 0:512],
                    start=(warm_state["n"] == 0),
                    stop=False,
                    skip_group_check=True,
                )
                warm_state["n"] += 1

            def flush_warm():
                if warm_state["ps"] is None:
                    return
                nc.tensor.matmul(
                    warm_state["ps"][:],
                    wqk_t[0][:, 0:128],
                    xt_t[0][:, 0:512],
                    start=False,
                    stop=True,
                    skip_group_check=True,
                )
                wsb = sbo.tile([1, 512], F32, name="warm_sb", tag="warm_sb")
                nc.vector.tensor_copy(wsb[:], warm_state["ps"][0:1, :])
                nc.sync.dma_start(warm[:, :], wsb[:])
                warm_state["ps"] = None

            p_tiles = {}    # (h, pa, tt) -> p tile
            accs = {}       # (h, pa) -> [2 psum accumulators]
            r_cs = {}       # (h, pa) -> [2 r tiles]
            stages = {}     # h -> B-head staging tile

            def retire(hp, pap, tpp):
                """Emit PV for t-tiles 2*tpp, 2*tpp+1; after the last pair
                emit the normalization tail inline."""
                h = hp
                if tpp == 0:
                    accs[(hp, pap)] = [
                        psa.tile([128, 512], F32, name=f"pv{hp}_{pap}_{cc}",
                                 tag="psa")
                        for cc in range(2)]
                acc = accs[(hp, pap)]
                for ttp in (2 * tpp, 2 * tpp + 1):
                    pt = p_tiles.pop((hp, pap, ttp))
                    for cc in range(2):
                        nc.tensor.matmul(
                            acc[cc][0:DH + 1, :],
                            v_t[ttp][:, h * (DH + 1):(h + 1) * (DH + 1)],
                            pt[:, cc * 512:(cc + 1) * 512],
                            start=(ttp == 0),
                            stop=(ttp == TT - 1),
                        )
                if tpp == TT // 2 - 1:
                    pr, half = hp // 2, hp % 2
                    if half == 0:
                        dst = att_t[pr]
                    else:
                        if pap == 0:
                            stages[hp] = sbst.tile([64, S], BF16,
                                                   name=f"bst{hp}",
                                                   tag="bstage")
                        dst = stages[hp]
                    for cc in range(2):
                        cg = pap * 2 + cc
                        # r row -> partition 0 of an sbuf tile, gpsimd
                        # broadcasts it to all 64 partitions (replaces a K=1
                        # PE matmul + its weight-register thrash)
                        rbc = sbr.tile([64, 512], F32,
                                       name=f"r{hp}_{pap}_{cc}", tag="r")
                        nc.vector.tensor_copy(rbc[0:1, :], acc[cc][64:65, :])
                        nc.gpsimd.partition_broadcast(rbc[:, :], rbc[0:1, :])
                        rinv = sbrv.tile([64, 512], F32,
                                         name=f"rinv{hp}_{pap}_{cc}",
                                         tag="rinv")
                        nc.vector.reciprocal_approx_fast(rinv[:], rbc[:])
                        nc.vector.tensor_mul(
                            dst[0:64, cg * 512:(cg + 1) * 512],
                            acc[cc][0:64, :],
                            rinv[:],
                        )
                    if half == 1 and pap == 1:
                        nc.sync.dma_start(att_t[pr][64:128, :],
                                          stages[hp][0:64, :])

            seq = [(h, pa, tp)
                   for h in head_order for pa in range(2)
                   for tp in range(TT // 2)]
            for i, (h, pa, tp) in enumerate(seq):
                qd, kd = dup_q[h], dup_k[h]
                tt0, tt1 = 2 * tp, 2 * tp + 1

                # emit PV/injection BEFORE the ST quad: the scheduler pops
                # ready work in priority (program) order, so nothing can
                # interleave into the row-packed A/B matmul pairs
                if pa == 1 and tp == 4:
                    # prefetch dup tiles for the next head (12 steps ahead;
                    # late enough that injected qkT writes precede the read)
                    hi = head_order.index(h)
                    if hi + 1 < len(head_order):
                        emit_dup(head_order[hi + 1])
                if i >= LAG:
                    retire(*seq[i - LAG])
                ran = 0
                while (inj_pos < len(inject) and inj_rel[inj_pos] <= i
                       and ran < 4):
                    inject[inj_pos]()
                    inj_pos += 1
                    ran += 1
                if ran == 0:
                    emit_warm()

                ptA = sbp.tile([128, 1024], BF16, name=f"p{h}_{pa}_{tt0}",
                               tag="p")
                ptB = sbp.tile([128, 1024], BF16, name=f"p{h}_{pa}_{tt1}",
                               tag="p")
                p_tiles[(h, pa, tt0)] = ptA
                p_tiles[(h, pa, tt1)] = ptB
                psA = psb.tile([128, 1024], F32, name=f"pstA{h}_{pa}_{tp}",
                               tag="psb")
                psB = psb.tile([128, 1024], F32, name=f"pstB{h}_{pa}_{tp}",
                               tag="psb")
                for j in range(2):
                    nc.tensor.matmul(
                        psA[:, j * 512:(j + 1) * 512],
                        kd[0:64, tt0 * 128:(tt0 + 1) * 128],
                        qd[0:64, pa * 1024 + j * 512:pa * 1024 + (j + 1) * 512],
                        start=True,
                        stop=True,
                    )
                    nc.tensor.matmul(
                        psB[:, j * 512:(j + 1) * 512],
                        kd[64:128, tt1 * 128:(tt1 + 1) * 128],
                        qd[64:128, pa * 1024 + j * 512:pa * 1024 + (j + 1) * 512],
                        start=True,
                        stop=True,
                    )
                nc.scalar.activation(
                    ptA[:], psA[:], mybir.ActivationFunctionType.Exp,
                    scale=0.125)
                nc.scalar.activation(
                    ptB[:], psB[:], mybir.ActivationFunctionType.Exp,
                    scale=0.125)
            while inj_pos < len(inject):
                inject[inj_pos]()
                inj_pos += 1
            for i in range(len(seq) - LAG, len(seq)):
                retire(*seq[i])
                for _ in range(6):
                    emit_warm()
            for _ in range(12):
                emit_warm()
            flush_warm()

            # ---------- phase 3: output projection ----------
            # attention pools are idle now; rotate psum groups across all
            # three pools for deeper pipelining
            for sb_i in range(TT):
                for e in range(2):
                    sel = (2 * sb_i + e) % 4
                    if sel < 2:
                        ps = psa.tile([128, 512], F32, name=f"po{sb_i}_{e}",
                                      tag="psa")
                    elif sel == 2:
                        psw = psb.tile([128, 1024], F32, name=f"po{sb_i}_{e}",
                                       tag="psb")
                        ps = psw[:, 0:512]
                    else:
                        ps = psc.tile([128, 512], F32, name=f"po{sb_i}_{e}",
                                      tag="psc")
                    for pr in range(NPAIR):
                        nc.tensor.matmul(
                            ps[:],
                            att_t[pr][:, sb_i * 128:(sb_i + 1) * 128],
                            wout_t[pr][:, e * 512:(e + 1) * 512],
                            start=(pr == 0),
                            stop=(pr == NPAIR - 1),
                        )
                    o = sbo.tile([128, 512], BF16, name=f"o{sb_i}_{e}", tag="o")
                    nc.vector.tensor_copy(o[:], ps[:])
                    nc.sync.dma_start(
                        out_p[sb_i * 128:(sb_i + 1) * 128,
                              e * 512:(e + 1) * 512], o[:])

    nc.compile()
    return nc


def get_nc():
    global _CACHED_NC
    if _CACHED_NC is None:
        _CACHED_NC = build_nc()
    return _CACHED_NC


def make_in_maps(x, w_qkv, w_out):
    bf = ml_dtypes.bfloat16
    in_maps = []
    for c in range(N_CORES):
        b = c // 2
        h0 = (c % 2) * NH
        q_cols = w_qkv[:, h0 * DH:(h0 + NH) * DH]
        k_cols = w_qkv[:, D + h0 * DH:D + (h0 + NH) * DH]
        v_cols = w_qkv[:, 2 * D + h0 * DH:2 * D + (h0 + NH) * DH]
        in_maps.append({
            "xt": np.ascontiguousarray(x[b].T).astype(bf),
            "wqk": np.concatenate([q_cols, k_cols], axis=1).astype(bf),
            "wv": np.ascontiguousarray(v_cols).astype(bf),
            "wout": np.ascontiguousarray(
                w_out[h0 * DH:(h0 + NH) * DH, :]).astype(bf),
        })
    return in_maps


def run(x, w_qkv, w_out, trace=False, trace_cores=None):
    nc = get_nc()
    in_maps = make_in_maps(x, w_qkv, w_out)
    res = bass_utils.run_bass_kernel_spmd(
        nc, in_maps, core_ids=list(range(N_CORES)),
        trace=trace, trace_cores=trace_cores,
    )
    partials = [np.asarray(res.results[c]["out_p"], dtype=np.float32)
                for c in range(N_CORES)]
    out = np.stack([partials[2 * b] + partials[2 * b + 1] for b in range(4)])
    return out.astype(np.float32), res


def kernel(x, w_qkv, w_out):
    out, _ = run(np.asarray(x), np.asarray(w_qkv), np.asarray(w_out))
    return out


if __name__ == "__main__":
    import tempfile
    nc = build_nc()
    with tempfile.TemporaryDirectory() as td:
        neff = bass_utils.compile_bass_kernel(nc, td)
        print("LOCAL COMPILE OK:", neff)
